# revision 23
# baseline (speedup 1.0000x reference)
"""Trainium2 Bass kernel for BasicCNN+LSTM (conv3x3+ReLU+GAP -> custom LSTM scan).

Self-contained: hardcodes shapes/sharding. Data-parallel over batch B=8 across
8 NeuronCores; each core processes one batch element end-to-end, the host
gathers the 8 [1,32] results.

Per-core device pipeline (per frame-pair g = frames 2g/2g+1):
  - DMA a host-prepacked, channel-deinterleaved "stack" [36, 56*112] bf16 per
    frame into SBUF bands at partitions {0, 64} (frame parity s).
  - Conv as K=36 matmuls (M=128: 2 px x 48 filters + 32 zero-pad cols to
    trigger the compiler's fast-weight-load), N=448 each. The two frames'
    matmuls are interleaved instruction-by-instruction so consecutive
    same-row-group matmuls are 2 apart: the PE pulls each LDWEIGHTS ahead
    during the other row group's stream, and the two streams run concurrently
    on the array (row groups h0/h64).
  - Outputs land on a persistent 6-bank PSUM ring [128, 3072]; frame-parity s
    owns slot parity s. Fused ReLU(+bias)+GAP drains cover up to 3 slots per
    instruction ([96, L, 448] stride-1024 views): ScalarE activation(Relu,
    accum_out) and VectorE tensor_scalar((x+bias) max 0, accum_out) (one
    tensor source = full DVE rate; the old scalar_tensor_tensor ran at half
    rate with two fp32 sources).
  - GAP finalize (reduce+add -> bf16 fsum) on the otherwise idle GpSimd.
  - Scan step in tanh-only form: sigmoid(x) = (tanh(x/2)+1)/2 folded into
    host-halved W1/W2 slots; one Tanh over all 96 gate cols + one Tanh(cell)
    on ScalarE; the elementwise recurrence runs on GpSimd against a halved
    hidden state (host doubles the output). Scan matmuls are bf16 (fp32 was
    4 cycles/row on the PE). The reference's state-order swap bug is kept.
"""
import sys
if '/opt/trn_rl_repo' not in sys.path:
    sys.path.insert(0, '/opt/trn_rl_repo')

import numpy as np
import ml_dtypes

import concourse.bass as bass
import concourse.mybir as mybir
import concourse.tile as tile
from concourse.vector_clock import ScopedClock
from concourse.bass_utils import run_bass_kernel_spmd

# ---------------------------------------------------------------- constants
B, T, H, W, C, F, U = 8, 24, 112, 112, 3, 48, 32
JA = 56            # vertical pixel-pair blocks (112 rows / 2)
KP = 36            # stack partitions: 3 c x 3 dx x 4 window rows
M = 96             # 2 pixels x 48 filters (real rows; stationary padded to 128)
NCHUNK = 14        # 448-col matmuls per frame
NQ = 448
FREE = JA * W      # stack free size per partition (elements)

FP32 = mybir.dt.float32
BF16 = mybir.dt.bfloat16

LAST_RESULTS = None  # BassKernelResults of the most recent run (for test.py)

# ------------------------------------------------- TileContext drain patch
# The container's walrus rejects >1 semaphore wait per instruction; Tile's
# kernel-tail drain aggregates all end-of-kernel waits onto one Drain.
# Spread them across single-wait NOPs on the sync engine instead.
def _patched_drain_and_barrier(self, tick_clock, wait_clock):
    nc = self.nc
    probe = nc.sync.nop(nofuse=True, hint="tail_waits")
    wait_clock.add_sem_waits(probe.ins, ScopedClock({None: tick_clock.global_clock}))
    waits = list(probe.ins.sync_info.on_wait or [])
    if len(waits) > 1:
        probe.ins.sync_info.on_wait = waits[:1]
        for i in range(1, len(waits)):
            extra = nc.sync.nop(nofuse=True, hint=f"tail_waits_{i}")
            si = extra.ins.sync_info
            if si is None:
                extra.ins.sync_info = mybir.SyncInfo(on_wait=[waits[i]], on_update=[])
            else:
                si.on_wait = [waits[i]]
    nc.sync.drain()
    nc.all_engine_barrier()
    popped = nc._tile_sem_poison_stack.pop()
    assert popped is self._sem_poison
    nc.clear_and_free_semaphores(list(self.sems.allocated().values()))
    nc.all_engine_barrier()


tile.TileContext._drain_and_barrier = _patched_drain_and_barrier

# Same walrus restriction for regular instructions: spill extra sem waits
# onto preceding same-engine NOPs at commit time.
_orig_commit = tile.TileContext._commit_instruction


def _patched_commit(self, inst, *args, **kwargs):
    si = getattr(inst, 'sync_info', None)
    if si is not None and si.on_wait and len(si.on_wait) > 1 \
            and inst.engine != mybir.EngineType.Unassigned:
        waits = list(si.on_wait)
        si.on_wait = waits[-1:]
        for w in waits[:-1]:
            nop = mybir.InstNoOp(
                name=self.nc.get_next_instruction_name(),
                ins=[], outs=[], bass_is_fusable=False)
            nop.engine = inst.engine
            nop.sync_info = mybir.SyncInfo(on_wait=[w], on_update=[])
            _orig_commit(self, nop, *args, **kwargs)
    return _orig_commit(self, inst, *args, **kwargs)


tile.TileContext._commit_instruction = _patched_commit

# NOTE: --enable-ldw-opt=true would dedupe the per-matmul stationary reloads,
# but this walrus build fails in visitInstLdweights with it enabled. Instead
# the matmul emission alternates PE row groups so each LDWEIGHTS is pulled
# ahead during the other group's stream.


# ------------------------------------------------------------- device code
def _build_bass(use_gbias=True, use_cbias=True):
    _build_bass.use_gbias = use_gbias
    _build_bass.use_cbias = use_cbias
    nc = bass.Bass('TRN2', target_bir_lowering=False, debug=False)

    xin = nc.dram_tensor('xin', [T, KP, FREE], BF16, kind='ExternalInput')
    smat_d = nc.dram_tensor('smat', [KP, 128], BF16, kind='ExternalInput')
    cbias_d = nc.dram_tensor('cbias', [M, 1], FP32, kind='ExternalInput')
    wfeat_d = nc.dram_tensor('wfeat', [M, 96], BF16, kind='ExternalInput')
    whid_d = nc.dram_tensor('whid', [U, 96], BF16, kind='ExternalInput')
    gbias_d = nc.dram_tensor('gbias', [1, 96], FP32, kind='ExternalInput')
    outh_d = nc.dram_tensor('outh', [1, U], FP32, kind='ExternalOutput')
    dbg_d = None
    if getattr(_build_bass, 'debug_fsums', False):
        dbg_d = nc.dram_tensor('dbg', [T, M], BF16, kind='ExternalOutput')

    Relu = mybir.ActivationFunctionType.Relu
    Tanh = mybir.ActivationFunctionType.Tanh
    Amax = mybir.AluOpType.max
    Aadd = mybir.AluOpType.add
    Amul = mybir.AluOpType.mult

    with tile.TileContext(nc) as tc:
        const = tc.alloc_tile_pool(name='const', bufs=1)
        state = tc.alloc_tile_pool(name='state', bufs=1)
        stackp = tc.alloc_tile_pool(name='stack', bufs=3)
        ringp = tc.alloc_tile_pool(name='ringp', bufs=1, space='PSUM')
        spsum = tc.alloc_tile_pool(name='spsum', bufs=2, space='PSUM')
        gs = tc.alloc_tile_pool(name='gs', bufs=4)
        fs = tc.alloc_tile_pool(name='fs', bufs=8)
        ga_pool = tc.alloc_tile_pool(name='ga', bufs=4)
        tmp = tc.alloc_tile_pool(name='tmp', bufs=8)

        # constants
        sc_all = const.tile([128, 128], BF16, tag='sc')
        for s in range(2):
            nc.sync.dma_start(sc_all[64 * s:64 * s + KP, :], smat_d[:])
        cbias = const.tile([M, 1], FP32, tag='cb')
        nc.sync.dma_start(cbias[:], cbias_d[:])
        wfeat = const.tile([M, 96], BF16, tag='wf')
        nc.sync.dma_start(wfeat[:], wfeat_d[:])
        whid = const.tile([U, 96], BF16, tag='wh')
        nc.sync.dma_start(whid[:], whid_d[:])
        gbias = const.tile([1, 96], FP32, tag='gb')
        nc.sync.dma_start(gbias[:], gbias_d[:])
        zt = const.tile([M, 3 * NQ], BF16, tag='zt')
        nc.vector.memset(zt[:], 0.0)
        ztv = zt.rearrange("p (r n) -> p r n", r=3)
        ones32 = const.tile([1, U], FP32, tag='ones32')
        nc.vector.memset(ones32[:], 1.0)
        halfs32 = const.tile([1, U], FP32, tag='halfs32')
        nc.vector.memset(halfs32[:], 0.5)

        # persistent scan state (hidden kept at half scale; host doubles out).
        # cellblk16 row 0 holds new_cell in bf16; a DVE 32x32 transpose gives
        # its partition-form in cpblk16's column 0 for the whid matmul.
        cellblk16 = state.tile([32, 32], BF16, tag='cellblk16')
        cpblk16 = state.tile([32, 32], BF16, tag='cpblk16')
        cellv = state.tile([1, U], FP32, tag='cellv')     # new_cell (fp32)
        cell_part = cpblk16[0:U, 0:1]                     # new_cell^T (bf16)
        hidh = state.tile([1, U], FP32, tag='hidh')       # 0.5 * new_hidden
        nc.vector.memset(cellblk16[:], 0.0)
        nc.vector.memset(cpblk16[:], 0.0)
        nc.vector.memset(cellv[:], 0.0)
        nc.vector.memset(hidh[:], 0.0)

        # 6-bank PSUM ring; all 14 matmuls of a frame cycle its 6 slots
        ring = ringp.tile([128, 3072], FP32, tag='ring')

        fsums = [None] * T

        frames = [None] * T

        def get_frame(t):
            # frame t split by column: cols 0-3135 -> band h0, rest -> h64
            if frames[t] is None:
                rt = stackp.tile([128, FREE // 2], BF16, tag='stk')
                nc.sync.dma_start(rt[0:KP, :], xin[t][:, 0:FREE // 2])
                nc.sync.dma_start(rt[64:64 + KP, :], xin[t][:, FREE // 2:])
                frames[t] = rt
            return frames[t]

        rview6 = ring[0:M, :].rearrange("p (b n) -> p b n", b=6)

        def emit_drain(slot0, nslots, gsum, col, eng):
            psv = rview6[:, slot0:slot0 + nslots, 0:NQ]
            if eng == 'A':
                nc.scalar.activation(psv, psv, Relu, bias=cbias[:],
                                     accum_out=gsum[:, col:col + 1])
            else:
                # (x + bias) max 0 with summing accum; bf16 zeros operand so
                # only one source is non-bf16 (full DVE rate)
                nc.vector.scalar_tensor_tensor(
                    out=psv, in0=psv, scalar=cbias[:],
                    in1=ztv[:, 0:nslots, :],
                    op0=Aadd, op1=Amax, accum_out=gsum[:, col:col + 1])

        # drains end after matmul k, covering ring slots [slot0, slot0+n);
        # 2-slot chunks on a 6-slot ring = depth 3: one pair filling while
        # both engines drain concurrently
        DRAINS = [(1, 0, 2), (3, 2, 2), (5, 4, 2), (7, 0, 2), (9, 2, 2),
                  (11, 4, 2), (13, 0, 2)]

        def emit_frame(t):
            rt = get_frame(t)
            if t + 2 < T:
                get_frame(t + 2)  # prefetch 2 frames ahead
            # alternate drain engines; flip per frame to balance 4/3 -> 3.5
            pat = ['A', 'D', 'A', 'D', 'A', 'D', 'A'] if t % 2 == 0 else \
                  ['D', 'A', 'D', 'A', 'D', 'A', 'D']
            gsumA = gs.tile([M, 4], FP32, tag='gsumA', name='gsumA')
            gsumB = gs.tile([M, 4], FP32, tag='gsumB', name='gsumB')
            cols = [0, 0]
            di = 0
            for k in range(NCHUNK):  # 14 matmuls
                b = k % 2           # band (row group)
                kc = k // 2         # 448-col chunk within the band
                band = rt[64 * b:64 * b + KP, :]
                lhsT = sc_all[64 * b:64 * b + KP, :]
                slot = k % 6
                nc.tensor.matmul(ring[:, slot * 512:slot * 512 + NQ], lhsT,
                                 band[:, kc * NQ:(kc + 1) * NQ],
                                 start=True, stop=True,
                                 tile_position=(64 * b, 0))
                if di < len(DRAINS) and k == DRAINS[di][0]:
                    eng = pat[di]
                    ei = 0 if eng == 'A' else 1
                    gsum = gsumA if eng == 'A' else gsumB
                    emit_drain(DRAINS[di][1], DRAINS[di][2], gsum,
                               cols[ei], eng)
                    cols[ei] += 1
                    di += 1

            # GAP finalize: pairwise add tree on the idle GpSimd -> bf16 fsum
            cs = [gsumA[:, c:c + 1] for c in range(cols[0])] + \
                 [gsumB[:, c:c + 1] for c in range(cols[1])]
            fsum = fs.tile([M, 1], BF16, tag='fsum')
            while len(cs) > 1:
                nxt = []
                for i in range(0, len(cs) - 1, 2):
                    o = fsum if len(cs) == 2 else \
                        tmp.tile([M, 1], FP32, tag='fst', name='fst')
                    nc.gpsimd.tensor_add(o[:], cs[i], cs[i + 1])
                    nxt.append(o)
                if len(cs) % 2:
                    nxt.append(cs[-1])
                cs = nxt
            fsums[t] = fsum
            if dbg_d is not None:
                nc.sync.dma_start(dbg_d[t], fsum[:])

        def emit_scan(t):
            # z-hidden part = prev new_cell (reference's state-order swap bug);
            # x1 multiplier = prev new_hidden (kept as hidh = hidden/2).
            fsum = fsums[t]
            pg = spsum.tile([1, 96], FP32, tag='sps')
            nc.tensor.matmul(pg[:], fsum[:], wfeat[:], start=True, stop=False)
            nc.tensor.matmul(pg[:], cell_part, whid[:], start=False, stop=True)
            if _build_bass.use_gbias:
                gpre = ga_pool.tile([1, 96], FP32, tag='gpre')
                nc.vector.tensor_add(gpre[:], pg[:], gbias[:])
            else:
                gpre = pg
            # gates in tanh form: W1/W2 slots were halved on the host, so
            # sigmoid(z@Wi) = (tanh(z@Wi/2)+1)/2 = (ga_i+1)/2.
            # elementwise recurrence on the otherwise idle GpSimd queue so it
            # never head-of-line blocks the conv drains on ACT/DVE
            ga = ga_pool.tile([1, 96], FP32, tag='ga')
            nc.scalar.activation(ga[:], gpre[:], Tanh)
            ua = tmp.tile([1, U], FP32, tag='ua')
            nc.gpsimd.tensor_add(ua[:], ga[:, 0:U], ones32[:])
            u = tmp.tile([1, U], FP32, tag='u')
            nc.gpsimd.tensor_mul(u[:], ua[:], hidh[:])    # = sig1 * prev_hid
            va = tmp.tile([1, U], FP32, tag='va')
            nc.gpsimd.tensor_add(va[:], ga[:, U:2 * U], ones32[:])
            vh = tmp.tile([1, U], FP32, tag='vh')
            nc.gpsimd.tensor_mul(vh[:], va[:], halfs32[:])
            v = tmp.tile([1, U], FP32, tag='v')
            nc.gpsimd.tensor_mul(v[:], vh[:], ga[:, 2 * U:3 * U])  # sig2*tanh3
            nc.gpsimd.tensor_add(cellv[:], v[:], u[:])    # new_cell
            tcl = tmp.tile([1, U], FP32, tag='tcl')
            nc.scalar.activation(tcl[:], cellv[:], Tanh)
            ch = tmp.tile([1, U], FP32, tag='ch')
            nc.gpsimd.tensor_mul(ch[:], cellv[:], halfs32[:])
            nc.gpsimd.tensor_mul(hidh[:], ch[:], tcl[:])  # new_hidden / 2
            if t < T - 1:
                nc.gpsimd.tensor_copy(cellblk16[0:1, :], cellv[:])
                nc.vector.transpose(cpblk16[:], cellblk16[:])

        LAG = 4  # frames of lag between a frame's conv and its scan step
        for t in range(T):
            emit_frame(t)
            if t >= LAG:
                emit_scan(t - LAG)
        for t in range(T - LAG, T):
            emit_scan(t)

        nc.sync.dma_start(outh_d[:], hidh[:])

        for p in (tmp, ga_pool, fs, gs, spsum, ringp, stackp, state, const):
            p.release()

    return nc


# -------------------------------------------------------------- host prep
def _prep_inputs(x, conv_w, conv_b, W1, b1, W2, b2, W3, b3):
    x = np.asarray(x, np.float32)
    conv_w = np.asarray(conv_w, np.float32)
    conv_b = np.asarray(conv_b, np.float32)

    xp = np.zeros((B, T, H + 2, W + 2, C), np.float32)
    xp[:, :, 1:H + 1, 1:W + 1, :] = x
    xin2 = np.empty((B, T, KP, JA, W), np.float32)
    rows = 2 * np.arange(JA)
    for c in range(3):
        for dx in range(3):
            for r in range(4):
                p = c * 12 + dx * 4 + r
                xin2[:, :, p] = np.moveaxis(
                    xp[:, :, rows + r, dx:dx + W, c], 0, 2)
    xin2 = xin2.reshape(B, T, KP, FREE).astype(ml_dtypes.bfloat16)

    smat = np.zeros((KP, 128), np.float32)
    for c in range(3):
        for dx in range(3):
            for r in range(4):
                p = c * 12 + dx * 4 + r
                for i in range(2):
                    dy = r - i
                    if 0 <= dy <= 2:
                        smat[p, i * F:(i + 1) * F] = conv_w[dy, dx, c, :]
    smat = smat.astype(ml_dtypes.bfloat16)
    cbias = np.concatenate([conv_b, conv_b]).reshape(M, 1).astype(np.float32)

    # gate weight layout [W1 | W2 | W3]; W1/W2 (sigmoid slots) halved for the
    # tanh-form sigmoid; wfeat rows also carry the GAP 1/(H*W).
    wfeat = np.zeros((M, 96), np.float32)
    whid = np.zeros((U, 96), np.float32)
    for g, Wg in enumerate([W1, W2, W3]):
        Wg = np.asarray(Wg, np.float32)
        half = 0.5 if g < 2 else 1.0
        for i in range(2):
            wfeat[i * F:(i + 1) * F, g * U:(g + 1) * U] = \
                Wg[0:F, :] * (half / float(H * W))
        whid[:, g * U:(g + 1) * U] = Wg[F:F + U, :] * half
    gbias = np.concatenate([
        np.asarray(b1, np.float32) * 0.5,
        np.asarray(b2, np.float32) * 0.5,
        np.asarray(b3, np.float32)]).reshape(1, 96)

    return (xin2, smat, cbias, wfeat.astype(ml_dtypes.bfloat16),
            whid.astype(ml_dtypes.bfloat16), gbias)


# ------------------------------------------------------------------ kernel
def kernel(x, conv_w, conv_b, W1, b1, W2, b2, W3, b3, W4, b4):
    global LAST_RESULTS
    xin2, smat, cbias, wfeat, whid, gbias = _prep_inputs(
        x, conv_w, conv_b, W1, b1, W2, b2, W3, b3)

    nc = _build_bass(use_gbias=bool(np.any(gbias)),
                     use_cbias=bool(np.any(cbias)))
    in_maps = [{
        'xin': np.ascontiguousarray(xin2[b]),
        'smat': smat,
        'cbias': cbias,
        'wfeat': wfeat,
        'whid': whid,
        'gbias': gbias,
    } for b in range(B)]

    res = run_bass_kernel_spmd(nc, in_maps, core_ids=list(range(B)))
    LAST_RESULTS = res
    out = np.stack([res.results[b]['outh'][0] for b in range(B)], axis=0)
    return (2.0 * out).astype(np.float32)


# revision 27
# speedup vs baseline: 2.0865x; 2.0865x over previous
"""Trainium2 Bass kernel for BasicCNN+LSTM (conv3x3+ReLU+GAP -> custom LSTM scan).

Self-contained: hardcodes shapes/sharding. Data-parallel over batch B=8 across
8 NeuronCores; each core processes one batch element end-to-end, the host
gathers the 8 [1,32] results.

Per-core device pipeline (per frame-pair g = frames 2g/2g+1):
  - DMA a host-prepacked, channel-deinterleaved "stack" [36, 56*112] bf16 per
    frame into SBUF bands at partitions {0, 64} (frame parity s).
  - Conv as K=36 matmuls (M=128: 2 px x 48 filters + 32 zero-pad cols to
    trigger the compiler's fast-weight-load), N=448 each. The two frames'
    matmuls are interleaved instruction-by-instruction so consecutive
    same-row-group matmuls are 2 apart: the PE pulls each LDWEIGHTS ahead
    during the other row group's stream, and the two streams run concurrently
    on the array (row groups h0/h64).
  - Outputs land on a persistent 6-bank PSUM ring [128, 3072]; frame-parity s
    owns slot parity s. Fused ReLU(+bias)+GAP drains cover up to 3 slots per
    instruction ([96, L, 448] stride-1024 views): ScalarE activation(Relu,
    accum_out) and VectorE tensor_scalar((x+bias) max 0, accum_out) (one
    tensor source = full DVE rate; the old scalar_tensor_tensor ran at half
    rate with two fp32 sources).
  - GAP finalize (reduce+add -> bf16 fsum) on the otherwise idle GpSimd.
  - Scan step in tanh-only form: sigmoid(x) = (tanh(x/2)+1)/2 folded into
    host-halved W1/W2 slots; one Tanh over all 96 gate cols + one Tanh(cell)
    on ScalarE; the elementwise recurrence runs on GpSimd against a halved
    hidden state (host doubles the output). Scan matmuls are bf16 (fp32 was
    4 cycles/row on the PE). The reference's state-order swap bug is kept.
"""
import sys
if '/opt/trn_rl_repo' not in sys.path:
    sys.path.insert(0, '/opt/trn_rl_repo')

import numpy as np
import ml_dtypes

import concourse.bass as bass
import concourse.mybir as mybir
import concourse.tile as tile
from concourse.vector_clock import ScopedClock
from concourse.bass_utils import run_bass_kernel_spmd

# ---------------------------------------------------------------- constants
B, T, H, W, C, F, U = 8, 24, 112, 112, 3, 48, 32
JA = 56            # vertical pixel-pair blocks (112 rows / 2)
KP = 36            # stack partitions: 3 c x 3 dx x 4 window rows
M = 96             # 2 pixels x 48 filters (real rows; stationary padded to 128)
NCHUNK = 14        # 448-col matmuls per frame
NQ = 448
FREE = JA * W      # stack free size per partition (elements)

FP32 = mybir.dt.float32
BF16 = mybir.dt.bfloat16

LAST_RESULTS = None  # BassKernelResults of the most recent run (for test.py)

# ------------------------------------------------- TileContext drain patch
# The container's walrus rejects >1 semaphore wait per instruction; Tile's
# kernel-tail drain aggregates all end-of-kernel waits onto one Drain.
# Spread them across single-wait NOPs on the sync engine instead.
def _patched_drain_and_barrier(self, tick_clock, wait_clock):
    nc = self.nc
    probe = nc.sync.nop(nofuse=True, hint="tail_waits")
    wait_clock.add_sem_waits(probe.ins, ScopedClock({None: tick_clock.global_clock}))
    waits = list(probe.ins.sync_info.on_wait or [])
    if len(waits) > 1:
        probe.ins.sync_info.on_wait = waits[:1]
        for i in range(1, len(waits)):
            extra = nc.sync.nop(nofuse=True, hint=f"tail_waits_{i}")
            si = extra.ins.sync_info
            if si is None:
                extra.ins.sync_info = mybir.SyncInfo(on_wait=[waits[i]], on_update=[])
            else:
                si.on_wait = [waits[i]]
    nc.sync.drain()
    nc.all_engine_barrier()
    popped = nc._tile_sem_poison_stack.pop()
    assert popped is self._sem_poison
    nc.clear_and_free_semaphores(list(self.sems.allocated().values()))
    nc.all_engine_barrier()


tile.TileContext._drain_and_barrier = _patched_drain_and_barrier

# Same walrus restriction for regular instructions: spill extra sem waits
# onto preceding same-engine NOPs at commit time.
_orig_commit = tile.TileContext._commit_instruction


def _patched_commit(self, inst, *args, **kwargs):
    si = getattr(inst, 'sync_info', None)
    if si is not None and si.on_wait and len(si.on_wait) > 1 \
            and inst.engine != mybir.EngineType.Unassigned:
        waits = list(si.on_wait)
        si.on_wait = waits[-1:]
        for w in waits[:-1]:
            nop = mybir.InstNoOp(
                name=self.nc.get_next_instruction_name(),
                ins=[], outs=[], bass_is_fusable=False)
            nop.engine = inst.engine
            nop.sync_info = mybir.SyncInfo(on_wait=[w], on_update=[])
            _orig_commit(self, nop, *args, **kwargs)
    return _orig_commit(self, inst, *args, **kwargs)


tile.TileContext._commit_instruction = _patched_commit

# NOTE: --enable-ldw-opt=true would dedupe the per-matmul stationary reloads,
# but this walrus build fails in visitInstLdweights with it enabled. Instead
# the matmul emission alternates PE row groups so each LDWEIGHTS is pulled
# ahead during the other group's stream.


# ------------------------------------------------------------- device code
def _build_bass(use_gbias=True, use_cbias=True):
    _build_bass.use_gbias = use_gbias
    _build_bass.use_cbias = use_cbias
    nc = bass.Bass('TRN2', target_bir_lowering=False, debug=False)

    xin = nc.dram_tensor('xin', [T, KP, FREE], BF16, kind='ExternalInput')
    smat_d = nc.dram_tensor('smat', [KP, 128], BF16, kind='ExternalInput')
    cbias_d = nc.dram_tensor('cbias', [M, 1], FP32, kind='ExternalInput')
    wfeat_d = nc.dram_tensor('wfeat', [M, 96], BF16, kind='ExternalInput')
    whid_d = nc.dram_tensor('whid', [U, 96], BF16, kind='ExternalInput')
    gbias_d = nc.dram_tensor('gbias', [1, 96], FP32, kind='ExternalInput')
    outh_d = nc.dram_tensor('outh', [1, U], FP32, kind='ExternalOutput')
    dbg_d = None
    if getattr(_build_bass, 'debug_fsums', False):
        dbg_d = nc.dram_tensor('dbg', [T, M], BF16, kind='ExternalOutput')

    Relu = mybir.ActivationFunctionType.Relu
    Tanh = mybir.ActivationFunctionType.Tanh
    Amax = mybir.AluOpType.max
    Aadd = mybir.AluOpType.add
    Amul = mybir.AluOpType.mult

    with tile.TileContext(nc) as tc:
        const = tc.alloc_tile_pool(name='const', bufs=1)
        state = tc.alloc_tile_pool(name='state', bufs=1)
        stackp = tc.alloc_tile_pool(name='stack', bufs=3)
        psum = tc.alloc_tile_pool(name='psum', bufs=3, space='PSUM')
        spsum = tc.alloc_tile_pool(name='spsum', bufs=2, space='PSUM')
        gs = tc.alloc_tile_pool(name='gs', bufs=4)
        fs = tc.alloc_tile_pool(name='fs', bufs=8)
        ga_pool = tc.alloc_tile_pool(name='ga', bufs=4)
        tmp = tc.alloc_tile_pool(name='tmp', bufs=8)

        # constants
        sc_all = const.tile([128, 128], BF16, tag='sc')
        for s in range(2):
            nc.sync.dma_start(sc_all[64 * s:64 * s + KP, :], smat_d[:])
        cbias = const.tile([M, 1], FP32, tag='cb')
        nc.sync.dma_start(cbias[:], cbias_d[:])
        wfeat = const.tile([M, 96], BF16, tag='wf')
        nc.sync.dma_start(wfeat[:], wfeat_d[:])
        whid = const.tile([U, 96], BF16, tag='wh')
        nc.sync.dma_start(whid[:], whid_d[:])
        gbias = const.tile([1, 96], FP32, tag='gb')
        nc.sync.dma_start(gbias[:], gbias_d[:])
        zt = const.tile([M, 3 * NQ], BF16, tag='zt')
        nc.vector.memset(zt[:], 0.0)
        ztv = zt.rearrange("p (r n) -> p r n", r=3)
        ones32 = const.tile([1, U], FP32, tag='ones32')
        nc.vector.memset(ones32[:], 1.0)
        halfs32 = const.tile([1, U], FP32, tag='halfs32')
        nc.vector.memset(halfs32[:], 0.5)

        # persistent scan state (hidden kept at half scale; host doubles out).
        # cellblk16 row 0 holds new_cell in bf16; a DVE 32x32 transpose gives
        # its partition-form in cpblk16's column 0 for the whid matmul.
        cellblk16 = state.tile([32, 32], BF16, tag='cellblk16')
        cpblk16 = state.tile([32, 32], BF16, tag='cpblk16')
        cellv = state.tile([1, U], FP32, tag='cellv')     # new_cell (fp32)
        cell_part = cpblk16[0:U, 0:1]                     # new_cell^T (bf16)
        hidh = state.tile([1, U], FP32, tag='hidh')       # 0.5 * new_hidden
        nc.vector.memset(cellblk16[:], 0.0)
        nc.vector.memset(cpblk16[:], 0.0)
        nc.vector.memset(cellv[:], 0.0)
        nc.vector.memset(hidh[:], 0.0)



        fsums = [None] * T

        frames = [None] * T

        def get_frame(t):
            # frame t split by column: cols 0-3135 -> band h0, rest -> h64
            if frames[t] is None:
                rt = stackp.tile([128, FREE // 2], BF16, tag='stk')
                nc.sync.dma_start(rt[0:KP, :], xin[t][:, 0:FREE // 2])
                nc.sync.dma_start(rt[64:64 + KP, :], xin[t][:, FREE // 2:])
                frames[t] = rt
            return frames[t]

        def emit_frame(t):
            rt = get_frame(t)
            if t + 2 < T:
                get_frame(t + 2)  # prefetch 2 frames ahead
            # alternate drain engines; flip per frame to balance 4/3 -> 3.5
            pat = 'ADADADA' if t % 2 == 0 else 'DADADAD'
            gsumA = gs.tile([M, 4], FP32, tag='gsumA', name='gsumA')
            gsumB = gs.tile([M, 4], FP32, tag='gsumB', name='gsumB')
            cols = [0, 0]
            for k7 in range(7):
                # one pool tile per 448-col chunk pair: band h0 chunk at
                # bank-aligned offset 0, band h64 chunk at 512; one drain
                ps = psum.tile([128, 1024], FP32, tag='ps')
                for b in range(2):
                    band = rt[64 * b:64 * b + KP, :]
                    lhsT = sc_all[64 * b:64 * b + KP, :]
                    nc.tensor.matmul(ps[:, b * 512:b * 512 + NQ], lhsT,
                                     band[:, k7 * NQ:(k7 + 1) * NQ],
                                     start=True, stop=True,
                                     tile_position=(64 * b, 0))
                psv = ps[0:M, :].rearrange("p (b n) -> p b n", b=2)[:, :, 0:NQ]
                eng = pat[k7]
                ei = 0 if eng == 'A' else 1
                gsum = gsumA if eng == 'A' else gsumB
                if eng == 'A':
                    nc.scalar.activation(psv, psv, Relu, bias=cbias[:],
                                         accum_out=gsum[:, cols[ei]:cols[ei] + 1])
                else:
                    # (x + bias) max 0 with summing accum; bf16 zeros operand
                    # so only one source is non-bf16 (full DVE rate)
                    nc.vector.scalar_tensor_tensor(
                        out=psv, in0=psv, scalar=cbias[:], in1=ztv[:, 0:2, :],
                        op0=Aadd, op1=Amax,
                        accum_out=gsum[:, cols[ei]:cols[ei] + 1])
                cols[ei] += 1

            # GAP finalize: pairwise add tree on the idle GpSimd -> bf16 fsum
            cs = [gsumA[:, c:c + 1] for c in range(cols[0])] + \
                 [gsumB[:, c:c + 1] for c in range(cols[1])]
            fsum = fs.tile([M, 1], BF16, tag='fsum')
            while len(cs) > 1:
                nxt = []
                for i in range(0, len(cs) - 1, 2):
                    o = fsum if len(cs) == 2 else \
                        tmp.tile([M, 1], FP32, tag='fst', name='fst')
                    nc.gpsimd.tensor_add(o[:], cs[i], cs[i + 1])
                    nxt.append(o)
                if len(cs) % 2:
                    nxt.append(cs[-1])
                cs = nxt
            fsums[t] = fsum
            if dbg_d is not None:
                nc.sync.dma_start(dbg_d[t], fsum[:])

        def emit_scan(t):
            # z-hidden part = prev new_cell (reference's state-order swap bug);
            # x1 multiplier = prev new_hidden (kept as hidh = hidden/2).
            fsum = fsums[t]
            pg = spsum.tile([1, 96], FP32, tag='sps')
            nc.tensor.matmul(pg[:], fsum[:], wfeat[:], start=True, stop=False)
            nc.tensor.matmul(pg[:], cell_part, whid[:], start=False, stop=True)
            if _build_bass.use_gbias:
                gpre = ga_pool.tile([1, 96], FP32, tag='gpre')
                nc.vector.tensor_add(gpre[:], pg[:], gbias[:])
            else:
                gpre = pg
            # gates in tanh form: W1/W2 slots were halved on the host, so
            # sigmoid(z@Wi) = (tanh(z@Wi/2)+1)/2 = (ga_i+1)/2.
            # elementwise recurrence on the otherwise idle GpSimd queue so it
            # never head-of-line blocks the conv drains on ACT/DVE
            ga = ga_pool.tile([1, 96], FP32, tag='ga')
            nc.scalar.activation(ga[:], gpre[:], Tanh)
            ua = tmp.tile([1, U], FP32, tag='ua')
            nc.gpsimd.tensor_add(ua[:], ga[:, 0:U], ones32[:])
            u = tmp.tile([1, U], FP32, tag='u')
            nc.gpsimd.tensor_mul(u[:], ua[:], hidh[:])    # = sig1 * prev_hid
            va = tmp.tile([1, U], FP32, tag='va')
            nc.gpsimd.tensor_add(va[:], ga[:, U:2 * U], ones32[:])
            vh = tmp.tile([1, U], FP32, tag='vh')
            nc.gpsimd.tensor_mul(vh[:], va[:], halfs32[:])
            v = tmp.tile([1, U], FP32, tag='v')
            nc.gpsimd.tensor_mul(v[:], vh[:], ga[:, 2 * U:3 * U])  # sig2*tanh3
            nc.gpsimd.tensor_add(cellv[:], v[:], u[:])    # new_cell
            tcl = tmp.tile([1, U], FP32, tag='tcl')
            nc.scalar.activation(tcl[:], cellv[:], Tanh)
            ch = tmp.tile([1, U], FP32, tag='ch')
            nc.gpsimd.tensor_mul(ch[:], cellv[:], halfs32[:])
            nc.gpsimd.tensor_mul(hidh[:], ch[:], tcl[:])  # new_hidden / 2
            if t < T - 1:
                nc.gpsimd.tensor_copy(cellblk16[0:1, :], cellv[:])
                nc.vector.transpose(cpblk16[:], cellblk16[:])

        LAG = 4  # frames of lag between a frame's conv and its scan step
        for t in range(T):
            emit_frame(t)
            if t >= LAG:
                emit_scan(t - LAG)
        for t in range(T - LAG, T):
            emit_scan(t)

        nc.sync.dma_start(outh_d[:], hidh[:])

        for p in (tmp, ga_pool, fs, gs, spsum, psum, stackp, state, const):
            p.release()

    return nc


# -------------------------------------------------------------- host prep
def _prep_inputs(x, conv_w, conv_b, W1, b1, W2, b2, W3, b3):
    x = np.asarray(x, np.float32)
    conv_w = np.asarray(conv_w, np.float32)
    conv_b = np.asarray(conv_b, np.float32)

    xp = np.zeros((B, T, H + 2, W + 2, C), np.float32)
    xp[:, :, 1:H + 1, 1:W + 1, :] = x
    xin2 = np.empty((B, T, KP, JA, W), np.float32)
    rows = 2 * np.arange(JA)
    for c in range(3):
        for dx in range(3):
            for r in range(4):
                p = c * 12 + dx * 4 + r
                xin2[:, :, p] = np.moveaxis(
                    xp[:, :, rows + r, dx:dx + W, c], 0, 2)
    xin2 = xin2.reshape(B, T, KP, FREE).astype(ml_dtypes.bfloat16)

    smat = np.zeros((KP, 128), np.float32)
    for c in range(3):
        for dx in range(3):
            for r in range(4):
                p = c * 12 + dx * 4 + r
                for i in range(2):
                    dy = r - i
                    if 0 <= dy <= 2:
                        smat[p, i * F:(i + 1) * F] = conv_w[dy, dx, c, :]
    smat = smat.astype(ml_dtypes.bfloat16)
    cbias = np.concatenate([conv_b, conv_b]).reshape(M, 1).astype(np.float32)

    # gate weight layout [W1 | W2 | W3]; W1/W2 (sigmoid slots) halved for the
    # tanh-form sigmoid; wfeat rows also carry the GAP 1/(H*W).
    wfeat = np.zeros((M, 96), np.float32)
    whid = np.zeros((U, 96), np.float32)
    for g, Wg in enumerate([W1, W2, W3]):
        Wg = np.asarray(Wg, np.float32)
        half = 0.5 if g < 2 else 1.0
        for i in range(2):
            wfeat[i * F:(i + 1) * F, g * U:(g + 1) * U] = \
                Wg[0:F, :] * (half / float(H * W))
        whid[:, g * U:(g + 1) * U] = Wg[F:F + U, :] * half
    gbias = np.concatenate([
        np.asarray(b1, np.float32) * 0.5,
        np.asarray(b2, np.float32) * 0.5,
        np.asarray(b3, np.float32)]).reshape(1, 96)

    return (xin2, smat, cbias, wfeat.astype(ml_dtypes.bfloat16),
            whid.astype(ml_dtypes.bfloat16), gbias)


# ------------------------------------------------------------------ kernel
def kernel(x, conv_w, conv_b, W1, b1, W2, b2, W3, b3, W4, b4):
    global LAST_RESULTS
    xin2, smat, cbias, wfeat, whid, gbias = _prep_inputs(
        x, conv_w, conv_b, W1, b1, W2, b2, W3, b3)

    nc = _build_bass(use_gbias=bool(np.any(gbias)),
                     use_cbias=bool(np.any(cbias)))
    in_maps = [{
        'xin': np.ascontiguousarray(xin2[b]),
        'smat': smat,
        'cbias': cbias,
        'wfeat': wfeat,
        'whid': whid,
        'gbias': gbias,
    } for b in range(B)]

    res = run_bass_kernel_spmd(nc, in_maps, core_ids=list(range(B)))
    LAST_RESULTS = res
    out = np.stack([res.results[b]['outh'][0] for b in range(B)], axis=0)
    return (2.0 * out).astype(np.float32)


# revision 29
# speedup vs baseline: 2.1190x; 1.0156x over previous
"""Trainium2 Bass kernel for BasicCNN+LSTM (conv3x3+ReLU+GAP -> custom LSTM scan).

Self-contained: hardcodes shapes/sharding. Data-parallel over batch B=8 across
8 NeuronCores; each core processes one batch element end-to-end, the host
gathers the 8 [1,32] results.

Per-core device pipeline (per frame-pair g = frames 2g/2g+1):
  - DMA a host-prepacked, channel-deinterleaved "stack" [36, 56*112] bf16 per
    frame into SBUF bands at partitions {0, 64} (frame parity s).
  - Conv as K=36 matmuls (M=128: 2 px x 48 filters + 32 zero-pad cols to
    trigger the compiler's fast-weight-load), N=448 each. The two frames'
    matmuls are interleaved instruction-by-instruction so consecutive
    same-row-group matmuls are 2 apart: the PE pulls each LDWEIGHTS ahead
    during the other row group's stream, and the two streams run concurrently
    on the array (row groups h0/h64).
  - Outputs land on a persistent 6-bank PSUM ring [128, 3072]; frame-parity s
    owns slot parity s. Fused ReLU(+bias)+GAP drains cover up to 3 slots per
    instruction ([96, L, 448] stride-1024 views): ScalarE activation(Relu,
    accum_out) and VectorE tensor_scalar((x+bias) max 0, accum_out) (one
    tensor source = full DVE rate; the old scalar_tensor_tensor ran at half
    rate with two fp32 sources).
  - GAP finalize (reduce+add -> bf16 fsum) on the otherwise idle GpSimd.
  - Scan step in tanh-only form: sigmoid(x) = (tanh(x/2)+1)/2 folded into
    host-halved W1/W2 slots; one Tanh over all 96 gate cols + one Tanh(cell)
    on ScalarE; the elementwise recurrence runs on GpSimd against a halved
    hidden state (host doubles the output). Scan matmuls are bf16 (fp32 was
    4 cycles/row on the PE). The reference's state-order swap bug is kept.
"""
import sys
if '/opt/trn_rl_repo' not in sys.path:
    sys.path.insert(0, '/opt/trn_rl_repo')

import numpy as np
import ml_dtypes

import concourse.bass as bass
import concourse.mybir as mybir
import concourse.tile as tile
from concourse.vector_clock import ScopedClock
from concourse.bass_utils import run_bass_kernel_spmd

# ---------------------------------------------------------------- constants
B, T, H, W, C, F, U = 8, 24, 112, 112, 3, 48, 32
JA = 56            # vertical pixel-pair blocks (112 rows / 2)
KP = 36            # stack partitions: 3 c x 3 dx x 4 window rows
M = 96             # 2 pixels x 48 filters (real rows; stationary padded to 128)
NCHUNK = 14        # 448-col matmuls per frame
NQ = 448
FREE = JA * W      # stack free size per partition (elements)

FP32 = mybir.dt.float32
BF16 = mybir.dt.bfloat16

LAST_RESULTS = None  # BassKernelResults of the most recent run (for test.py)

# ------------------------------------------------- TileContext drain patch
# The container's walrus rejects >1 semaphore wait per instruction; Tile's
# kernel-tail drain aggregates all end-of-kernel waits onto one Drain.
# Spread them across single-wait NOPs on the sync engine instead.
def _patched_drain_and_barrier(self, tick_clock, wait_clock):
    nc = self.nc
    probe = nc.sync.nop(nofuse=True, hint="tail_waits")
    wait_clock.add_sem_waits(probe.ins, ScopedClock({None: tick_clock.global_clock}))
    waits = list(probe.ins.sync_info.on_wait or [])
    if len(waits) > 1:
        probe.ins.sync_info.on_wait = waits[:1]
        for i in range(1, len(waits)):
            extra = nc.sync.nop(nofuse=True, hint=f"tail_waits_{i}")
            si = extra.ins.sync_info
            if si is None:
                extra.ins.sync_info = mybir.SyncInfo(on_wait=[waits[i]], on_update=[])
            else:
                si.on_wait = [waits[i]]
    nc.sync.drain()
    nc.all_engine_barrier()
    popped = nc._tile_sem_poison_stack.pop()
    assert popped is self._sem_poison
    nc.clear_and_free_semaphores(list(self.sems.allocated().values()))
    nc.all_engine_barrier()


tile.TileContext._drain_and_barrier = _patched_drain_and_barrier

# Same walrus restriction for regular instructions: spill extra sem waits
# onto preceding same-engine NOPs at commit time.
_orig_commit = tile.TileContext._commit_instruction


def _patched_commit(self, inst, *args, **kwargs):
    si = getattr(inst, 'sync_info', None)
    if si is not None and si.on_wait and len(si.on_wait) > 1 \
            and inst.engine != mybir.EngineType.Unassigned:
        waits = list(si.on_wait)
        si.on_wait = waits[-1:]
        for w in waits[:-1]:
            nop = mybir.InstNoOp(
                name=self.nc.get_next_instruction_name(),
                ins=[], outs=[], bass_is_fusable=False)
            nop.engine = inst.engine
            nop.sync_info = mybir.SyncInfo(on_wait=[w], on_update=[])
            _orig_commit(self, nop, *args, **kwargs)
    return _orig_commit(self, inst, *args, **kwargs)


tile.TileContext._commit_instruction = _patched_commit

# NOTE: --enable-ldw-opt=true would dedupe the per-matmul stationary reloads,
# but this walrus build fails in visitInstLdweights with it enabled. Instead
# the matmul emission alternates PE row groups so each LDWEIGHTS is pulled
# ahead during the other group's stream.


# ------------------------------------------------------------- device code
def _build_bass(use_gbias=True, use_cbias=True):
    _build_bass.use_gbias = use_gbias
    _build_bass.use_cbias = use_cbias
    nc = bass.Bass('TRN2', target_bir_lowering=False, debug=False)

    xin = nc.dram_tensor('xin', [T, KP, FREE], BF16, kind='ExternalInput')
    smat_d = nc.dram_tensor('smat', [KP, 128], BF16, kind='ExternalInput')
    cbias_d = nc.dram_tensor('cbias', [M, 1], FP32, kind='ExternalInput')
    wfeat_d = nc.dram_tensor('wfeat', [M, 96], BF16, kind='ExternalInput')
    whid_d = nc.dram_tensor('whid', [U, 96], BF16, kind='ExternalInput')
    gbias_d = nc.dram_tensor('gbias', [1, 96], FP32, kind='ExternalInput')
    outh_d = nc.dram_tensor('outh', [1, U], FP32, kind='ExternalOutput')
    dbg_d = None
    if getattr(_build_bass, 'debug_fsums', False):
        dbg_d = nc.dram_tensor('dbg', [T, M], BF16, kind='ExternalOutput')

    Relu = mybir.ActivationFunctionType.Relu
    Tanh = mybir.ActivationFunctionType.Tanh
    Amax = mybir.AluOpType.max
    Aadd = mybir.AluOpType.add
    Amul = mybir.AluOpType.mult

    with tile.TileContext(nc) as tc:
        const = tc.alloc_tile_pool(name='const', bufs=1)
        state = tc.alloc_tile_pool(name='state', bufs=1)
        stackp = tc.alloc_tile_pool(name='stack', bufs=3)
        psum = tc.alloc_tile_pool(name='psum', bufs=3, space='PSUM')
        spsum = tc.alloc_tile_pool(name='spsum', bufs=2, space='PSUM')
        gs = tc.alloc_tile_pool(name='gs', bufs=4)
        fs = tc.alloc_tile_pool(name='fs', bufs=8)
        ga_pool = tc.alloc_tile_pool(name='ga', bufs=4)
        tmp = tc.alloc_tile_pool(name='tmp', bufs=8)

        # constants
        sc_all = const.tile([128, 128], BF16, tag='sc')
        for s in range(2):
            nc.sync.dma_start(sc_all[64 * s:64 * s + KP, :], smat_d[:])
        cbias = const.tile([M, 1], FP32, tag='cb')
        nc.sync.dma_start(cbias[:], cbias_d[:])
        wfeat = const.tile([M, 96], BF16, tag='wf')
        nc.sync.dma_start(wfeat[:], wfeat_d[:])
        whid = const.tile([U, 96], BF16, tag='wh')
        nc.sync.dma_start(whid[:], whid_d[:])
        gbias = const.tile([1, 96], FP32, tag='gb')
        nc.sync.dma_start(gbias[:], gbias_d[:])
        zt = const.tile([M, 3 * NQ], BF16, tag='zt')
        nc.vector.memset(zt[:], 0.0)
        ztv = zt.rearrange("p (r n) -> p r n", r=3)
        ones2 = const.tile([1, 2 * U], FP32, tag='ones2')
        nc.vector.memset(ones2[:], 1.0)
        halfs32 = const.tile([1, U], FP32, tag='halfs32')
        nc.vector.memset(halfs32[:], 0.5)

        # persistent scan state (hidden kept at half scale; host doubles out).
        # cellblk16 row 0 holds new_cell in bf16; a DVE 32x32 transpose gives
        # its partition-form in cpblk16's column 0 for the whid matmul.
        cellblk16 = state.tile([32, 32], BF16, tag='cellblk16')
        cpblk16 = state.tile([32, 32], BF16, tag='cpblk16')
        cellv = state.tile([1, U], FP32, tag='cellv')     # new_cell (fp32)
        cell_part = cpblk16[0:U, 0:1]                     # new_cell^T (bf16)
        hidh = state.tile([1, U], FP32, tag='hidh')       # 0.5 * new_hidden
        nc.vector.memset(cellblk16[:], 0.0)
        nc.vector.memset(cpblk16[:], 0.0)
        nc.vector.memset(cellv[:], 0.0)
        nc.vector.memset(hidh[:], 0.0)



        fsums = [None] * T

        frames = [None] * T

        def get_frame(t):
            # frame t split by column: cols 0-3135 -> band h0, rest -> h64
            if frames[t] is None:
                rt = stackp.tile([128, FREE // 2], BF16, tag='stk')
                nc.sync.dma_start(rt[0:KP, :], xin[t][:, 0:FREE // 2])
                nc.sync.dma_start(rt[64:64 + KP, :], xin[t][:, FREE // 2:])
                frames[t] = rt
            return frames[t]

        def emit_tile(rt, k7, eng, gsum, col):
            # one pool tile per 448-col chunk pair: band h0 chunk at
            # bank-aligned offset 0, band h64 chunk at 512; one drain
            ps = psum.tile([128, 1024], FP32, tag='ps', name='ps')
            for b in range(2):
                band = rt[64 * b:64 * b + KP, :]
                lhsT = sc_all[64 * b:64 * b + KP, :]
                nc.tensor.matmul(ps[:, b * 512:b * 512 + NQ], lhsT,
                                 band[:, k7 * NQ:(k7 + 1) * NQ],
                                 start=True, stop=True,
                                 tile_position=(64 * b, 0))
            psv = ps[0:M, :].rearrange("p (b n) -> p b n", b=2)[:, :, 0:NQ]
            if eng == 'A':
                nc.scalar.activation(psv, psv, Relu, bias=cbias[:],
                                     accum_out=gsum[:, col:col + 1])
            elif _build_bass.use_cbias:
                # (x + bias) max 0 with summing accum (two tensor sources)
                nc.vector.scalar_tensor_tensor(
                    out=psv, in0=psv, scalar=cbias[:], in1=ztv[:, 0:2, :],
                    op0=Aadd, op1=Amax, accum_out=gsum[:, col:col + 1])
            else:
                # zero bias: (x max 0) add 0; accum reduces with op1 (add),
                # single tensor source -> full DVE rate
                nc.vector.tensor_scalar(
                    out=psv, in0=psv, scalar1=0.0, scalar2=0.0,
                    op0=Amax, op1=Aadd, accum_out=gsum[:, col:col + 1])

        def emit_finalize(t, gsumA, gsumB, cols):
            # GAP finalize: pairwise add tree on the idle GpSimd -> bf16 fsum
            cs = [gsumA[:, c:c + 1] for c in range(cols[0])] + \
                 [gsumB[:, c:c + 1] for c in range(cols[1])]
            fsum = fs.tile([M, 1], BF16, tag='fsum', name='fsum')
            while len(cs) > 1:
                nxt = []
                for i in range(0, len(cs) - 1, 2):
                    o = fsum if len(cs) == 2 else \
                        tmp.tile([M, 1], FP32, tag='fst', name='fst')
                    nc.gpsimd.tensor_add(o[:], cs[i], cs[i + 1])
                    nxt.append(o)
                if len(cs) % 2:
                    nxt.append(cs[-1])
                cs = nxt
            fsums[t] = fsum
            if dbg_d is not None:
                nc.sync.dma_start(dbg_d[t], fsum[:])

        gas = [None] * T

        def emit_scan_a(t):
            # z-hidden part = prev new_cell (reference's state-order swap bug);
            # x1 multiplier = prev new_hidden (kept as hidh = hidden/2).
            fsum = fsums[t]
            pg = spsum.tile([1, 96], FP32, tag='sps')
            nc.tensor.matmul(pg[:], fsum[:], wfeat[:], start=True, stop=False)
            nc.tensor.matmul(pg[:], cell_part, whid[:], start=False, stop=True)
            if _build_bass.use_gbias:
                gpre = ga_pool.tile([1, 96], FP32, tag='gpre')
                nc.vector.tensor_add(gpre[:], pg[:], gbias[:])
            else:
                gpre = pg
            # gates in tanh form: W1/W2 slots were halved on the host, so
            # sigmoid(z@Wi) = (tanh(z@Wi/2)+1)/2 = (ga_i+1)/2.
            # elementwise recurrence on the otherwise idle GpSimd queue so it
            # never head-of-line blocks the conv drains on ACT/DVE
            ga = ga_pool.tile([1, 96], FP32, tag='ga')
            nc.scalar.activation(ga[:], gpre[:], Tanh)
            gas[t] = ga
            ua = tmp.tile([1, 2 * U], FP32, tag='ua')
            nc.gpsimd.tensor_add(ua[:], ga[:, 0:2 * U], ones2[:])
            u = tmp.tile([1, U], FP32, tag='u')
            nc.gpsimd.tensor_mul(u[:], ua[:, 0:U], hidh[:])  # = sig1*prev_hid
            vh = tmp.tile([1, U], FP32, tag='vh')
            nc.gpsimd.tensor_mul(vh[:], ua[:, U:2 * U], halfs32[:])
            v = tmp.tile([1, U], FP32, tag='v')
            nc.gpsimd.tensor_mul(v[:], vh[:], ga[:, 2 * U:3 * U])  # sig2*tanh3
            nc.gpsimd.tensor_add(cellv[:], v[:], u[:])    # new_cell

        def emit_scan_b(t):
            tcl = tmp.tile([1, U], FP32, tag='tcl')
            nc.scalar.activation(tcl[:], cellv[:], Tanh)
            ch = tmp.tile([1, U], FP32, tag='ch')
            nc.gpsimd.tensor_mul(ch[:], cellv[:], halfs32[:])
            nc.gpsimd.tensor_mul(hidh[:], ch[:], tcl[:])  # new_hidden / 2
            if t < T - 1:
                nc.gpsimd.tensor_copy(cellblk16[0:1, :], cellv[:])
                nc.vector.transpose(cpblk16[:], cellblk16[:])

        LAG = 4  # frames of lag between a frame's conv and its scan step
        for t in range(T):
            rt = get_frame(t)
            if t + 2 < T:
                get_frame(t + 2)  # prefetch 2 frames ahead
            # alternate drain engines; flip per frame to balance 4/3 -> 3.5
            pat = 'ADADADA' if t % 2 == 0 else 'DADADAD'
            gsumA = gs.tile([M, 4], FP32, tag='gsumA', name='gsumA')
            gsumB = gs.tile([M, 4], FP32, tag='gsumB', name='gsumB')
            cols = [0, 0]
            for k7 in range(7):
                eng = pat[k7]
                ei = 0 if eng == 'A' else 1
                emit_tile(rt, k7, eng, gsumA if eng == 'A' else gsumB,
                          cols[ei])
                cols[ei] += 1
                # stagger the scan step so each of its cross-engine hops
                # enters its FIFO with ~2 conv tiles of dep slack
                if k7 == 2 and t >= LAG:
                    emit_scan_a(t - LAG)
                if k7 == 5 and t >= LAG:
                    emit_scan_b(t - LAG)
            emit_finalize(t, gsumA, gsumB, cols)
        for t in range(T - LAG, T):
            emit_scan_a(t)
            emit_scan_b(t)

        nc.sync.dma_start(outh_d[:], hidh[:])

        for p in (tmp, ga_pool, fs, gs, spsum, psum, stackp, state, const):
            p.release()

    return nc


# -------------------------------------------------------------- host prep
def _prep_inputs(x, conv_w, conv_b, W1, b1, W2, b2, W3, b3):
    x = np.asarray(x, np.float32)
    conv_w = np.asarray(conv_w, np.float32)
    conv_b = np.asarray(conv_b, np.float32)

    xp = np.zeros((B, T, H + 2, W + 2, C), np.float32)
    xp[:, :, 1:H + 1, 1:W + 1, :] = x
    xin2 = np.empty((B, T, KP, JA, W), np.float32)
    rows = 2 * np.arange(JA)
    for c in range(3):
        for dx in range(3):
            for r in range(4):
                p = c * 12 + dx * 4 + r
                xin2[:, :, p] = np.moveaxis(
                    xp[:, :, rows + r, dx:dx + W, c], 0, 2)
    xin2 = xin2.reshape(B, T, KP, FREE).astype(ml_dtypes.bfloat16)

    smat = np.zeros((KP, 128), np.float32)
    for c in range(3):
        for dx in range(3):
            for r in range(4):
                p = c * 12 + dx * 4 + r
                for i in range(2):
                    dy = r - i
                    if 0 <= dy <= 2:
                        smat[p, i * F:(i + 1) * F] = conv_w[dy, dx, c, :]
    smat = smat.astype(ml_dtypes.bfloat16)
    cbias = np.concatenate([conv_b, conv_b]).reshape(M, 1).astype(np.float32)

    # gate weight layout [W1 | W2 | W3]; W1/W2 (sigmoid slots) halved for the
    # tanh-form sigmoid; wfeat rows also carry the GAP 1/(H*W).
    wfeat = np.zeros((M, 96), np.float32)
    whid = np.zeros((U, 96), np.float32)
    for g, Wg in enumerate([W1, W2, W3]):
        Wg = np.asarray(Wg, np.float32)
        half = 0.5 if g < 2 else 1.0
        for i in range(2):
            wfeat[i * F:(i + 1) * F, g * U:(g + 1) * U] = \
                Wg[0:F, :] * (half / float(H * W))
        whid[:, g * U:(g + 1) * U] = Wg[F:F + U, :] * half
    gbias = np.concatenate([
        np.asarray(b1, np.float32) * 0.5,
        np.asarray(b2, np.float32) * 0.5,
        np.asarray(b3, np.float32)]).reshape(1, 96)

    return (xin2, smat, cbias, wfeat.astype(ml_dtypes.bfloat16),
            whid.astype(ml_dtypes.bfloat16), gbias)


# ------------------------------------------------------------------ kernel
def kernel(x, conv_w, conv_b, W1, b1, W2, b2, W3, b3, W4, b4):
    global LAST_RESULTS
    xin2, smat, cbias, wfeat, whid, gbias = _prep_inputs(
        x, conv_w, conv_b, W1, b1, W2, b2, W3, b3)

    nc = _build_bass(use_gbias=bool(np.any(gbias)),
                     use_cbias=bool(np.any(cbias)))
    in_maps = [{
        'xin': np.ascontiguousarray(xin2[b]),
        'smat': smat,
        'cbias': cbias,
        'wfeat': wfeat,
        'whid': whid,
        'gbias': gbias,
    } for b in range(B)]

    res = run_bass_kernel_spmd(nc, in_maps, core_ids=list(range(B)))
    LAST_RESULTS = res
    out = np.stack([res.results[b]['outh'][0] for b in range(B)], axis=0)
    return (2.0 * out).astype(np.float32)


# revision 30
# speedup vs baseline: 2.1224x; 1.0016x over previous
"""Trainium2 Bass kernel for BasicCNN+LSTM (conv3x3+ReLU+GAP -> custom LSTM scan).

Self-contained: hardcodes shapes/sharding. Data-parallel over batch B=8 across
8 NeuronCores; each core processes one batch element end-to-end, the host
gathers the 8 [1,32] results.

Per-core device pipeline (per frame-pair g = frames 2g/2g+1):
  - DMA a host-prepacked, channel-deinterleaved "stack" [36, 56*112] bf16 per
    frame into SBUF bands at partitions {0, 64} (frame parity s).
  - Conv as K=36 matmuls (M=128: 2 px x 48 filters + 32 zero-pad cols to
    trigger the compiler's fast-weight-load), N=448 each. The two frames'
    matmuls are interleaved instruction-by-instruction so consecutive
    same-row-group matmuls are 2 apart: the PE pulls each LDWEIGHTS ahead
    during the other row group's stream, and the two streams run concurrently
    on the array (row groups h0/h64).
  - Outputs land on a persistent 6-bank PSUM ring [128, 3072]; frame-parity s
    owns slot parity s. Fused ReLU(+bias)+GAP drains cover up to 3 slots per
    instruction ([96, L, 448] stride-1024 views): ScalarE activation(Relu,
    accum_out) and VectorE tensor_scalar((x+bias) max 0, accum_out) (one
    tensor source = full DVE rate; the old scalar_tensor_tensor ran at half
    rate with two fp32 sources).
  - GAP finalize (reduce+add -> bf16 fsum) on the otherwise idle GpSimd.
  - Scan step in tanh-only form: sigmoid(x) = (tanh(x/2)+1)/2 folded into
    host-halved W1/W2 slots; one Tanh over all 96 gate cols + one Tanh(cell)
    on ScalarE; the elementwise recurrence runs on GpSimd against a halved
    hidden state (host doubles the output). Scan matmuls are bf16 (fp32 was
    4 cycles/row on the PE). The reference's state-order swap bug is kept.
"""
import sys
if '/opt/trn_rl_repo' not in sys.path:
    sys.path.insert(0, '/opt/trn_rl_repo')

import numpy as np
import ml_dtypes

import concourse.bass as bass
import concourse.mybir as mybir
import concourse.tile as tile
from concourse.vector_clock import ScopedClock
from concourse.bass_utils import run_bass_kernel_spmd

# ---------------------------------------------------------------- constants
B, T, H, W, C, F, U = 8, 24, 112, 112, 3, 48, 32
JA = 56            # vertical pixel-pair blocks (112 rows / 2)
KP = 36            # stack partitions: 3 c x 3 dx x 4 window rows
M = 96             # 2 pixels x 48 filters (real rows; stationary padded to 128)
NCHUNK = 14        # 448-col matmuls per frame
NQ = 448
FREE = JA * W      # stack free size per partition (elements)

FP32 = mybir.dt.float32
BF16 = mybir.dt.bfloat16

LAST_RESULTS = None  # BassKernelResults of the most recent run (for test.py)

# ------------------------------------------------- TileContext drain patch
# The container's walrus rejects >1 semaphore wait per instruction; Tile's
# kernel-tail drain aggregates all end-of-kernel waits onto one Drain.
# Spread them across single-wait NOPs on the sync engine instead.
def _patched_drain_and_barrier(self, tick_clock, wait_clock):
    nc = self.nc
    probe = nc.sync.nop(nofuse=True, hint="tail_waits")
    wait_clock.add_sem_waits(probe.ins, ScopedClock({None: tick_clock.global_clock}))
    waits = list(probe.ins.sync_info.on_wait or [])
    if len(waits) > 1:
        probe.ins.sync_info.on_wait = waits[:1]
        for i in range(1, len(waits)):
            extra = nc.sync.nop(nofuse=True, hint=f"tail_waits_{i}")
            si = extra.ins.sync_info
            if si is None:
                extra.ins.sync_info = mybir.SyncInfo(on_wait=[waits[i]], on_update=[])
            else:
                si.on_wait = [waits[i]]
    nc.sync.drain()
    nc.all_engine_barrier()
    popped = nc._tile_sem_poison_stack.pop()
    assert popped is self._sem_poison
    nc.clear_and_free_semaphores(list(self.sems.allocated().values()))
    nc.all_engine_barrier()


tile.TileContext._drain_and_barrier = _patched_drain_and_barrier

# Same walrus restriction for regular instructions: spill extra sem waits
# onto preceding same-engine NOPs at commit time.
_orig_commit = tile.TileContext._commit_instruction


def _patched_commit(self, inst, *args, **kwargs):
    si = getattr(inst, 'sync_info', None)
    if si is not None and si.on_wait and len(si.on_wait) > 1 \
            and inst.engine != mybir.EngineType.Unassigned:
        waits = list(si.on_wait)
        si.on_wait = waits[-1:]
        for w in waits[:-1]:
            nop = mybir.InstNoOp(
                name=self.nc.get_next_instruction_name(),
                ins=[], outs=[], bass_is_fusable=False)
            nop.engine = inst.engine
            nop.sync_info = mybir.SyncInfo(on_wait=[w], on_update=[])
            _orig_commit(self, nop, *args, **kwargs)
    return _orig_commit(self, inst, *args, **kwargs)


tile.TileContext._commit_instruction = _patched_commit

# NOTE: --enable-ldw-opt=true would dedupe the per-matmul stationary reloads,
# but this walrus build fails in visitInstLdweights with it enabled. Instead
# the matmul emission alternates PE row groups so each LDWEIGHTS is pulled
# ahead during the other group's stream.


# ------------------------------------------------------------- device code
def _build_bass(use_gbias=True, use_cbias=True):
    _build_bass.use_gbias = use_gbias
    _build_bass.use_cbias = use_cbias
    nc = bass.Bass('TRN2', target_bir_lowering=False, debug=False)

    xin = nc.dram_tensor('xin', [T, KP, FREE], BF16, kind='ExternalInput')
    smat_d = nc.dram_tensor('smat', [KP, 128], BF16, kind='ExternalInput')
    cbias_d = nc.dram_tensor('cbias', [M, 1], FP32, kind='ExternalInput')
    wfeat_d = nc.dram_tensor('wfeat', [M, 96], BF16, kind='ExternalInput')
    whid_d = nc.dram_tensor('whid', [U, 96], BF16, kind='ExternalInput')
    gbias_d = nc.dram_tensor('gbias', [1, 96], FP32, kind='ExternalInput')
    outh_d = nc.dram_tensor('outh', [1, U], FP32, kind='ExternalOutput')
    dbg_d = None
    if getattr(_build_bass, 'debug_fsums', False):
        dbg_d = nc.dram_tensor('dbg', [T, M], BF16, kind='ExternalOutput')

    Relu = mybir.ActivationFunctionType.Relu
    Tanh = mybir.ActivationFunctionType.Tanh
    Amax = mybir.AluOpType.max
    Aadd = mybir.AluOpType.add
    Amul = mybir.AluOpType.mult

    with tile.TileContext(nc) as tc:
        const = tc.alloc_tile_pool(name='const', bufs=1)
        state = tc.alloc_tile_pool(name='state', bufs=1)
        stackp = tc.alloc_tile_pool(name='stack', bufs=3)
        psum = tc.alloc_tile_pool(name='psum', bufs=3, space='PSUM')
        spsum = tc.alloc_tile_pool(name='spsum', bufs=2, space='PSUM')
        gs = tc.alloc_tile_pool(name='gs', bufs=4)
        fs = tc.alloc_tile_pool(name='fs', bufs=8)
        ga_pool = tc.alloc_tile_pool(name='ga', bufs=4)
        tmp = tc.alloc_tile_pool(name='tmp', bufs=8)

        # constants
        sc_all = const.tile([128, 128], BF16, tag='sc')
        for s in range(2):
            nc.sync.dma_start(sc_all[64 * s:64 * s + KP, :], smat_d[:])
        cbias = const.tile([M, 1], FP32, tag='cb')
        nc.sync.dma_start(cbias[:], cbias_d[:])
        wfeat = const.tile([M, 96], BF16, tag='wf')
        nc.sync.dma_start(wfeat[:], wfeat_d[:])
        whid = const.tile([U, 96], BF16, tag='wh')
        nc.sync.dma_start(whid[:], whid_d[:])
        gbias = const.tile([1, 96], FP32, tag='gb')
        nc.sync.dma_start(gbias[:], gbias_d[:])
        zt = const.tile([M, 3 * NQ], BF16, tag='zt')
        nc.vector.memset(zt[:], 0.0)
        ztv = zt.rearrange("p (r n) -> p r n", r=3)
        ones2 = const.tile([1, 2 * U], FP32, tag='ones2')
        nc.vector.memset(ones2[:], 1.0)
        halfs32 = const.tile([1, U], FP32, tag='halfs32')
        nc.vector.memset(halfs32[:], 0.5)

        # persistent scan state (hidden kept at half scale; host doubles out).
        # cellblk16 row 0 holds new_cell in bf16; a DVE 32x32 transpose gives
        # its partition-form in cpblk16's column 0 for the whid matmul.
        cellblk16 = state.tile([32, 32], BF16, tag='cellblk16')
        cpblk16 = state.tile([32, 32], BF16, tag='cpblk16')
        cellv = state.tile([1, U], FP32, tag='cellv')     # new_cell (fp32)
        cell_part = cpblk16[0:U, 0:1]                     # new_cell^T (bf16)
        hidh = state.tile([1, U], FP32, tag='hidh')       # 0.5 * new_hidden
        nc.vector.memset(cellblk16[:], 0.0)
        nc.vector.memset(cpblk16[:], 0.0)
        nc.vector.memset(cellv[:], 0.0)
        nc.vector.memset(hidh[:], 0.0)



        fsums = [None] * T

        frames = [None] * T

        def get_frame(t):
            # frame t split by column: cols 0-3135 -> band h0, rest -> h64
            if frames[t] is None:
                rt = stackp.tile([128, FREE // 2], BF16, tag='stk')
                nc.sync.dma_start(rt[0:KP, :], xin[t][:, 0:FREE // 2])
                nc.sync.dma_start(rt[64:64 + KP, :], xin[t][:, FREE // 2:])
                frames[t] = rt
            return frames[t]

        def emit_tile(rt, k7, eng, gsum, col):
            # one pool tile per 448-col chunk pair: band h0 chunk at
            # bank-aligned offset 0, band h64 chunk at 512; one drain
            ps = psum.tile([128, 1024], FP32, tag='ps', name='ps')
            for b in range(2):
                band = rt[64 * b:64 * b + KP, :]
                lhsT = sc_all[64 * b:64 * b + KP, :]
                nc.tensor.matmul(ps[:, b * 512:b * 512 + NQ], lhsT,
                                 band[:, k7 * NQ:(k7 + 1) * NQ],
                                 start=True, stop=True,
                                 tile_position=(64 * b, 0))
            psv = ps[0:M, :].rearrange("p (b n) -> p b n", b=2)[:, :, 0:NQ]
            if eng == 'A':
                nc.scalar.activation(psv, psv, Relu, bias=cbias[:],
                                     accum_out=gsum[:, col:col + 1])
            elif _build_bass.use_cbias:
                # (x + bias) max 0 with summing accum (two tensor sources)
                nc.vector.scalar_tensor_tensor(
                    out=psv, in0=psv, scalar=cbias[:], in1=ztv[:, 0:2, :],
                    op0=Aadd, op1=Amax, accum_out=gsum[:, col:col + 1])
            else:
                # zero bias: (x max 0) add 0; accum reduces with op1 (add),
                # single tensor source -> full DVE rate
                nc.vector.tensor_scalar(
                    out=psv, in0=psv, scalar1=0.0, scalar2=0.0,
                    op0=Amax, op1=Aadd, accum_out=gsum[:, col:col + 1])

        def emit_finalize(t, gsumA, gsumB, cols):
            # GAP finalize: pairwise add tree on the idle GpSimd -> bf16 fsum
            cs = [gsumA[:, c:c + 1] for c in range(cols[0])] + \
                 [gsumB[:, c:c + 1] for c in range(cols[1])]
            fsum = fs.tile([M, 1], BF16, tag='fsum', name='fsum')
            while len(cs) > 1:
                nxt = []
                for i in range(0, len(cs) - 1, 2):
                    o = fsum if len(cs) == 2 else \
                        tmp.tile([M, 1], FP32, tag='fst', name='fst')
                    nc.gpsimd.tensor_add(o[:], cs[i], cs[i + 1])
                    nxt.append(o)
                if len(cs) % 2:
                    nxt.append(cs[-1])
                cs = nxt
            fsums[t] = fsum
            if dbg_d is not None:
                nc.sync.dma_start(dbg_d[t], fsum[:])

        gas = [None] * T

        def emit_scan_a(t):
            # z-hidden part = prev new_cell (reference's state-order swap bug);
            # x1 multiplier = prev new_hidden (kept as hidh = hidden/2).
            fsum = fsums[t]
            pg = spsum.tile([1, 96], FP32, tag='sps')
            nc.tensor.matmul(pg[:], fsum[:], wfeat[:], start=True, stop=False)
            nc.tensor.matmul(pg[:], cell_part, whid[:], start=False, stop=True)
            if _build_bass.use_gbias:
                gpre = ga_pool.tile([1, 96], FP32, tag='gpre')
                nc.vector.tensor_add(gpre[:], pg[:], gbias[:])
            else:
                gpre = pg
            # gates in tanh form: W1/W2 slots were halved on the host, so
            # sigmoid(z@Wi) = (tanh(z@Wi/2)+1)/2 = (ga_i+1)/2.
            # elementwise recurrence on the otherwise idle GpSimd queue so it
            # never head-of-line blocks the conv drains on ACT/DVE
            ga = ga_pool.tile([1, 96], FP32, tag='ga')
            nc.scalar.activation(ga[:], gpre[:], Tanh)
            gas[t] = ga
            ua = tmp.tile([1, 2 * U], FP32, tag='ua')
            nc.gpsimd.tensor_add(ua[:], ga[:, 0:2 * U], ones2[:])
            u = tmp.tile([1, U], FP32, tag='u')
            nc.gpsimd.tensor_mul(u[:], ua[:, 0:U], hidh[:])  # = sig1*prev_hid
            vh = tmp.tile([1, U], FP32, tag='vh')
            nc.gpsimd.tensor_mul(vh[:], ua[:, U:2 * U], halfs32[:])
            v = tmp.tile([1, U], FP32, tag='v')
            nc.gpsimd.tensor_mul(v[:], vh[:], ga[:, 2 * U:3 * U])  # sig2*tanh3
            nc.gpsimd.tensor_add(cellv[:], v[:], u[:])    # new_cell

        def emit_scan_b(t):
            tcl = tmp.tile([1, U], FP32, tag='tcl')
            nc.scalar.activation(tcl[:], cellv[:], Tanh)
            ch = tmp.tile([1, U], FP32, tag='ch')
            nc.gpsimd.tensor_mul(ch[:], cellv[:], halfs32[:])
            nc.gpsimd.tensor_mul(hidh[:], ch[:], tcl[:])  # new_hidden / 2
            if t < T - 1:
                nc.gpsimd.tensor_copy(cellblk16[0:1, :], cellv[:])
                nc.vector.transpose(cpblk16[:], cellblk16[:])

        LAG = 6  # frames of lag between a frame's conv and its scan step
        for t in range(T):
            rt = get_frame(t)
            if t + 2 < T:
                get_frame(t + 2)  # prefetch 2 frames ahead
            # alternate drain engines; flip per frame to balance 4/3 -> 3.5
            pat = 'ADADADA' if t % 2 == 0 else 'DADADAD'
            gsumA = gs.tile([M, 4], FP32, tag='gsumA', name='gsumA')
            gsumB = gs.tile([M, 4], FP32, tag='gsumB', name='gsumB')
            cols = [0, 0]
            for k7 in range(7):
                eng = pat[k7]
                ei = 0 if eng == 'A' else 1
                emit_tile(rt, k7, eng, gsumA if eng == 'A' else gsumB,
                          cols[ei])
                cols[ei] += 1
                # stagger the scan step so each of its cross-engine hops
                # enters its FIFO with ~2 conv tiles of dep slack
                if k7 == 1 and t >= LAG:
                    emit_scan_a(t - LAG)
                if k7 == 4 and t >= LAG:
                    emit_scan_b(t - LAG)
            emit_finalize(t, gsumA, gsumB, cols)
        for t in range(T - LAG, T):
            emit_scan_a(t)
            emit_scan_b(t)

        nc.sync.dma_start(outh_d[:], hidh[:])

        for p in (tmp, ga_pool, fs, gs, spsum, psum, stackp, state, const):
            p.release()

    return nc


# -------------------------------------------------------------- host prep
def _prep_inputs(x, conv_w, conv_b, W1, b1, W2, b2, W3, b3):
    x = np.asarray(x, np.float32)
    conv_w = np.asarray(conv_w, np.float32)
    conv_b = np.asarray(conv_b, np.float32)

    xp = np.zeros((B, T, H + 2, W + 2, C), np.float32)
    xp[:, :, 1:H + 1, 1:W + 1, :] = x
    xin2 = np.empty((B, T, KP, JA, W), np.float32)
    rows = 2 * np.arange(JA)
    for c in range(3):
        for dx in range(3):
            for r in range(4):
                p = c * 12 + dx * 4 + r
                xin2[:, :, p] = np.moveaxis(
                    xp[:, :, rows + r, dx:dx + W, c], 0, 2)
    xin2 = xin2.reshape(B, T, KP, FREE).astype(ml_dtypes.bfloat16)

    smat = np.zeros((KP, 128), np.float32)
    for c in range(3):
        for dx in range(3):
            for r in range(4):
                p = c * 12 + dx * 4 + r
                for i in range(2):
                    dy = r - i
                    if 0 <= dy <= 2:
                        smat[p, i * F:(i + 1) * F] = conv_w[dy, dx, c, :]
    smat = smat.astype(ml_dtypes.bfloat16)
    cbias = np.concatenate([conv_b, conv_b]).reshape(M, 1).astype(np.float32)

    # gate weight layout [W1 | W2 | W3]; W1/W2 (sigmoid slots) halved for the
    # tanh-form sigmoid; wfeat rows also carry the GAP 1/(H*W).
    wfeat = np.zeros((M, 96), np.float32)
    whid = np.zeros((U, 96), np.float32)
    for g, Wg in enumerate([W1, W2, W3]):
        Wg = np.asarray(Wg, np.float32)
        half = 0.5 if g < 2 else 1.0
        for i in range(2):
            wfeat[i * F:(i + 1) * F, g * U:(g + 1) * U] = \
                Wg[0:F, :] * (half / float(H * W))
        whid[:, g * U:(g + 1) * U] = Wg[F:F + U, :] * half
    gbias = np.concatenate([
        np.asarray(b1, np.float32) * 0.5,
        np.asarray(b2, np.float32) * 0.5,
        np.asarray(b3, np.float32)]).reshape(1, 96)

    return (xin2, smat, cbias, wfeat.astype(ml_dtypes.bfloat16),
            whid.astype(ml_dtypes.bfloat16), gbias)


# ------------------------------------------------------------------ kernel
def kernel(x, conv_w, conv_b, W1, b1, W2, b2, W3, b3, W4, b4):
    global LAST_RESULTS
    xin2, smat, cbias, wfeat, whid, gbias = _prep_inputs(
        x, conv_w, conv_b, W1, b1, W2, b2, W3, b3)

    nc = _build_bass(use_gbias=bool(np.any(gbias)),
                     use_cbias=bool(np.any(cbias)))
    in_maps = [{
        'xin': np.ascontiguousarray(xin2[b]),
        'smat': smat,
        'cbias': cbias,
        'wfeat': wfeat,
        'whid': whid,
        'gbias': gbias,
    } for b in range(B)]

    res = run_bass_kernel_spmd(nc, in_maps, core_ids=list(range(B)))
    LAST_RESULTS = res
    out = np.stack([res.results[b]['outh'][0] for b in range(B)], axis=0)
    return (2.0 * out).astype(np.float32)


# revision 45
# speedup vs baseline: 2.1274x; 1.0024x over previous
"""Trainium2 Bass kernel for BasicCNN+LSTM (conv3x3+ReLU+GAP -> custom LSTM scan).

Self-contained: hardcodes shapes/sharding. Data-parallel over batch B=8 across
8 NeuronCores; each core processes one batch element end-to-end, the host
gathers the 8 [1,32] results.

Per-core device pipeline (per frame t of 24):
  - The frame's host-prepacked stack [36, 6272] bf16 is split by column into
    two SBUF bands (cols 0-3135 at partitions 0-35, rest at 64-99), so both
    PE row groups (h0/h64) work on ONE frame: the two matmuls of each tile
    run concurrently on the array and every LDWEIGHTS is pulled ahead during
    the other row group's stream (walrus' ldw-opt dedupe is broken, so the
    reload must be hidden, not removed).
  - Conv as K=36, M=128 matmuls (stationary zero-padded from 96 to 128 cols
    to trigger the compiler's fast-weight-load), N=448, into pool-rotated
    2-bank PSUM tiles [128, 1024] (bufs=3; per-tile pool rotation is what
    makes the WAR deps tile-granular - a single manually-slotted PSUM ring
    serialized everything through coarse subtile deps).
  - One fused ReLU(+bias)+GAP drain per tile over the [96, 2, 448] view,
    alternating engines per tile: ScalarE activation(Relu, accum_out) and
    VectorE tensor_scalar((x max 0) add 0, accum_out) - accum reduces with
    op1, so op1 must be the add; a single tensor source keeps full DVE rate.
  - GAP finalize (pairwise add tree -> bf16 fsum) on the otherwise idle
    GpSimd (Pool) engine; walrus rejects TensorScalarPtr on Pool, so only
    tensor_tensor/tcopy run there.
  - Scan step in tanh-only form: sigmoid(x) = (tanh(x/2)+1)/2 folded into
    host-halved W1/W2 slots; one Tanh over all 96 gate cols + one Tanh(cell)
    on ScalarE; the elementwise recurrence runs on GpSimd against a halved
    hidden state (host doubles the output), so it never head-of-line blocks
    conv drains on ACT/DVE; the step is emitted in two staggered halves so
    each cross-engine hop enters its FIFO with ~2 conv tiles of dep slack.
    Scan matmuls are bf16 (fp32 is 4 cycles/row on the PE). The reference's
    state-order swap bug is reproduced faithfully.

Known dead ends (measured): 3-slot drains on a 6-slot manual ring serialize
(subtile deps go coarse + only depth-2); matmuls into PE column-quadrant 3
(tile_position col 96) crash the exec unit (HW bug per docs); GpSimd has no
PSUM port; tensor_tensor_reduce fails walrus codegen.
"""
import sys
if '/opt/trn_rl_repo' not in sys.path:
    sys.path.insert(0, '/opt/trn_rl_repo')

import numpy as np
import ml_dtypes

import concourse.bass as bass
import concourse.mybir as mybir
import concourse.tile as tile
from concourse.vector_clock import ScopedClock
from concourse.bass_utils import run_bass_kernel_spmd

# ---------------------------------------------------------------- constants
B, T, H, W, C, F, U = 8, 24, 112, 112, 3, 48, 32
JA = 56            # vertical pixel-pair blocks (112 rows / 2)
KP = 36            # stack partitions: 3 c x 3 dx x 4 window rows
M = 96             # 2 pixels x 48 filters (real rows; stationary padded to 128)
NCHUNK = 14        # 448-col matmuls per frame
NQ = 448
FREE = JA * W      # stack free size per partition (elements)

FP32 = mybir.dt.float32
BF16 = mybir.dt.bfloat16

LAST_RESULTS = None  # BassKernelResults of the most recent run (for test.py)

# ------------------------------------------------- TileContext drain patch
# The container's walrus rejects >1 semaphore wait per instruction; Tile's
# kernel-tail drain aggregates all end-of-kernel waits onto one Drain.
# Spread them across single-wait NOPs on the sync engine instead.
def _patched_drain_and_barrier(self, tick_clock, wait_clock):
    nc = self.nc
    probe = nc.sync.nop(nofuse=True, hint="tail_waits")
    wait_clock.add_sem_waits(probe.ins, ScopedClock({None: tick_clock.global_clock}))
    waits = list(probe.ins.sync_info.on_wait or [])
    if len(waits) > 1:
        probe.ins.sync_info.on_wait = waits[:1]
        for i in range(1, len(waits)):
            extra = nc.sync.nop(nofuse=True, hint=f"tail_waits_{i}")
            si = extra.ins.sync_info
            if si is None:
                extra.ins.sync_info = mybir.SyncInfo(on_wait=[waits[i]], on_update=[])
            else:
                si.on_wait = [waits[i]]
    nc.sync.drain()
    nc.all_engine_barrier()
    popped = nc._tile_sem_poison_stack.pop()
    assert popped is self._sem_poison
    nc.clear_and_free_semaphores(list(self.sems.allocated().values()))
    nc.all_engine_barrier()


tile.TileContext._drain_and_barrier = _patched_drain_and_barrier

# Same walrus restriction for regular instructions: spill extra sem waits
# onto preceding same-engine NOPs at commit time.
_orig_commit = tile.TileContext._commit_instruction


def _patched_commit(self, inst, *args, **kwargs):
    si = getattr(inst, 'sync_info', None)
    if si is not None and si.on_wait and len(si.on_wait) > 1 \
            and inst.engine != mybir.EngineType.Unassigned:
        waits = list(si.on_wait)
        si.on_wait = waits[-1:]
        for w in waits[:-1]:
            nop = mybir.InstNoOp(
                name=self.nc.get_next_instruction_name(),
                ins=[], outs=[], bass_is_fusable=False)
            nop.engine = inst.engine
            nop.sync_info = mybir.SyncInfo(on_wait=[w], on_update=[])
            _orig_commit(self, nop, *args, **kwargs)
    return _orig_commit(self, inst, *args, **kwargs)


tile.TileContext._commit_instruction = _patched_commit

# NOTE: --enable-ldw-opt=true would dedupe the per-matmul stationary reloads,
# but this walrus build fails in visitInstLdweights with it enabled. Instead
# the matmul emission alternates PE row groups so each LDWEIGHTS is pulled
# ahead during the other group's stream.


# ------------------------------------------------------------- device code
def _build_bass(use_gbias=True, use_cbias=True):
    _build_bass.use_gbias = use_gbias
    _build_bass.use_cbias = use_cbias
    nc = bass.Bass('TRN2', target_bir_lowering=False, debug=False)

    xin = nc.dram_tensor('xin', [T, KP, FREE], BF16, kind='ExternalInput')
    smat_d = nc.dram_tensor('smat', [KP, 128], BF16, kind='ExternalInput')
    cbias_d = nc.dram_tensor('cbias', [M, 1], FP32, kind='ExternalInput')
    wfeat_d = nc.dram_tensor('wfeat', [M, 96], BF16, kind='ExternalInput')
    whid_d = nc.dram_tensor('whid', [U, 96], BF16, kind='ExternalInput')
    gbias_d = nc.dram_tensor('gbias', [1, 96], FP32, kind='ExternalInput')
    outh_d = nc.dram_tensor('outh', [1, U], FP32, kind='ExternalOutput')
    dbg_d = None
    if getattr(_build_bass, 'debug_fsums', False):
        dbg_d = nc.dram_tensor('dbg', [T, M], BF16, kind='ExternalOutput')

    Relu = mybir.ActivationFunctionType.Relu
    Tanh = mybir.ActivationFunctionType.Tanh
    Amax = mybir.AluOpType.max
    Aadd = mybir.AluOpType.add
    Amul = mybir.AluOpType.mult

    with tile.TileContext(nc) as tc:
        const = tc.alloc_tile_pool(name='const', bufs=1)
        state = tc.alloc_tile_pool(name='state', bufs=1)
        stackp = tc.alloc_tile_pool(name='stack', bufs=3)
        psum = tc.alloc_tile_pool(name='psum', bufs=3, space='PSUM')
        spsum = tc.alloc_tile_pool(name='spsum', bufs=2, space='PSUM')
        gs = tc.alloc_tile_pool(name='gs', bufs=4)
        fs = tc.alloc_tile_pool(name='fs', bufs=8)
        ga_pool = tc.alloc_tile_pool(name='ga', bufs=4)
        tmp = tc.alloc_tile_pool(name='tmp', bufs=8)

        # constants
        sc_all = const.tile([128, 128], BF16, tag='sc')
        for s in range(2):
            nc.sync.dma_start(sc_all[64 * s:64 * s + KP, :], smat_d[:])
        cbias = const.tile([M, 1], FP32, tag='cb')
        nc.sync.dma_start(cbias[:], cbias_d[:])
        wfeat = const.tile([M, 96], BF16, tag='wf')
        nc.sync.dma_start(wfeat[:], wfeat_d[:])
        whid = const.tile([U, 96], BF16, tag='wh')
        nc.sync.dma_start(whid[:], whid_d[:])
        gbias = const.tile([1, 96], FP32, tag='gb')
        nc.sync.dma_start(gbias[:], gbias_d[:])
        zt = const.tile([M, 3 * NQ], BF16, tag='zt')
        nc.vector.memset(zt[:], 0.0)
        ztv = zt.rearrange("p (r n) -> p r n", r=3)
        ones2 = const.tile([1, 2 * U], FP32, tag='ones2')
        nc.vector.memset(ones2[:], 1.0)
        halfs32 = const.tile([1, U], FP32, tag='halfs32')
        nc.vector.memset(halfs32[:], 0.5)

        # persistent scan state (hidden kept at half scale; host doubles out).
        # cellblk16 row 0 holds new_cell in bf16; a DVE 32x32 transpose gives
        # its partition-form in cpblk16's column 0 for the whid matmul.
        cellblk16 = state.tile([32, 32], BF16, tag='cellblk16')
        cpblk16 = state.tile([32, 32], BF16, tag='cpblk16')
        cellv = state.tile([1, U], FP32, tag='cellv')     # new_cell (fp32)
        cell_part = cpblk16[0:U, 0:1]                     # new_cell^T (bf16)
        hidh = state.tile([1, U], FP32, tag='hidh')       # 0.5 * new_hidden
        nc.vector.memset(cellblk16[:], 0.0)
        nc.vector.memset(cpblk16[:], 0.0)
        nc.vector.memset(cellv[:], 0.0)
        nc.vector.memset(hidh[:], 0.0)



        fsums = [None] * T

        frames = [None] * T

        def get_frame(t):
            # frame t split by column: cols 0-3135 -> band h0, rest -> h64
            if frames[t] is None:
                rt = stackp.tile([128, FREE // 2], BF16, tag='stk')
                nc.sync.dma_start(rt[0:KP, :], xin[t][:, 0:FREE // 2])
                nc.sync.dma_start(rt[64:64 + KP, :], xin[t][:, FREE // 2:])
                frames[t] = rt
            return frames[t]

        def emit_tile(rt, k7, eng, gsum, col):
            # one pool tile per 448-col chunk pair: band h0 chunk at
            # bank-aligned offset 0, band h64 chunk at 512; one drain
            ps = psum.tile([128, 1024], FP32, tag='ps', name='ps')
            for b in range(2):
                band = rt[64 * b:64 * b + KP, :]
                lhsT = sc_all[64 * b:64 * b + KP, :]
                nc.tensor.matmul(ps[:, b * 512:b * 512 + NQ], lhsT,
                                 band[:, k7 * NQ:(k7 + 1) * NQ],
                                 start=True, stop=True,
                                 tile_position=(64 * b, 0))
            psv = ps[0:M, :].rearrange("p (b n) -> p b n", b=2)[:, :, 0:NQ]
            if eng == 'A':
                nc.scalar.activation(psv, psv, Relu, bias=cbias[:],
                                     accum_out=gsum[:, col:col + 1])
            elif _build_bass.use_cbias:
                # (x + bias) max 0 with summing accum (two tensor sources)
                nc.vector.scalar_tensor_tensor(
                    out=psv, in0=psv, scalar=cbias[:], in1=ztv[:, 0:2, :],
                    op0=Aadd, op1=Amax, accum_out=gsum[:, col:col + 1])
            else:
                # zero bias: (x max 0) add 0; accum reduces with op1 (add),
                # single tensor source -> full DVE rate
                nc.vector.tensor_scalar(
                    out=psv, in0=psv, scalar1=0.0, scalar2=0.0,
                    op0=Amax, op1=Aadd, accum_out=gsum[:, col:col + 1])

        def emit_finalize(t, gsumA, gsumB, cols):
            # GAP finalize: pairwise add tree on the idle GpSimd -> bf16 fsum
            cs = [gsumA[:, c:c + 1] for c in range(cols[0])] + \
                 [gsumB[:, c:c + 1] for c in range(cols[1])]
            fsum = fs.tile([M, 1], BF16, tag='fsum', name='fsum')
            while len(cs) > 1:
                nxt = []
                for i in range(0, len(cs) - 1, 2):
                    o = fsum if len(cs) == 2 else \
                        tmp.tile([M, 1], FP32, tag='fst', name='fst')
                    nc.gpsimd.tensor_add(o[:], cs[i], cs[i + 1])
                    nxt.append(o)
                if len(cs) % 2:
                    nxt.append(cs[-1])
                cs = nxt
            fsums[t] = fsum
            if dbg_d is not None:
                nc.sync.dma_start(dbg_d[t], fsum[:])

        def emit_scan_a(t):
            # z-hidden part = prev new_cell (reference's state-order swap bug);
            # x1 multiplier = prev new_hidden (kept as hidh = hidden/2).
            fsum = fsums[t]
            pg = spsum.tile([1, 96], FP32, tag='sps')
            nc.tensor.matmul(pg[:], fsum[:], wfeat[:], start=True, stop=False)
            nc.tensor.matmul(pg[:], cell_part, whid[:], start=False, stop=True)
            if _build_bass.use_gbias:
                gpre = ga_pool.tile([1, 96], FP32, tag='gpre')
                nc.vector.tensor_add(gpre[:], pg[:], gbias[:])
            else:
                gpre = pg
            # gates in tanh form: W1/W2 slots were halved on the host, so
            # sigmoid(z@Wi) = (tanh(z@Wi/2)+1)/2 = (ga_i+1)/2.
            # elementwise recurrence on the otherwise idle GpSimd queue so it
            # never head-of-line blocks the conv drains on ACT/DVE
            ga = ga_pool.tile([1, 96], FP32, tag='ga')
            nc.scalar.activation(ga[:], gpre[:], Tanh)
            ua = tmp.tile([1, 2 * U], FP32, tag='ua')
            nc.gpsimd.tensor_add(ua[:], ga[:, 0:2 * U], ones2[:])
            u = tmp.tile([1, U], FP32, tag='u')
            nc.gpsimd.tensor_mul(u[:], ua[:, 0:U], hidh[:])  # = sig1*prev_hid
            vh = tmp.tile([1, U], FP32, tag='vh')
            nc.gpsimd.tensor_mul(vh[:], ua[:, U:2 * U], halfs32[:])
            v = tmp.tile([1, U], FP32, tag='v')
            nc.gpsimd.tensor_mul(v[:], vh[:], ga[:, 2 * U:3 * U])  # sig2*tanh3
            nc.gpsimd.tensor_add(cellv[:], v[:], u[:])    # new_cell

        def emit_scan_b(t):
            tcl = tmp.tile([1, U], FP32, tag='tcl')
            nc.scalar.activation(tcl[:], cellv[:], Tanh)
            ch = tmp.tile([1, U], FP32, tag='ch')
            nc.gpsimd.tensor_mul(ch[:], cellv[:], halfs32[:])
            nc.gpsimd.tensor_mul(hidh[:], ch[:], tcl[:])  # new_hidden / 2
            if t < T - 1:
                nc.gpsimd.tensor_copy(cellblk16[0:1, :], cellv[:])
                nc.vector.transpose(cpblk16[:], cellblk16[:])

        LAG = 6  # frames of lag between a frame's conv and its scan step
        for t in range(T):
            rt = get_frame(t)
            if t + 2 < T:
                get_frame(t + 2)  # prefetch 2 frames ahead
            # alternate drain engines; flip per frame to balance 4/3 -> 3.5
            pat = 'ADADADA' if t % 2 == 0 else 'DADADAD'
            gsumA = gs.tile([M, 4], FP32, tag='gsumA', name='gsumA')
            gsumB = gs.tile([M, 4], FP32, tag='gsumB', name='gsumB')
            cols = [0, 0]
            for k7 in range(7):
                eng = pat[k7]
                ei = 0 if eng == 'A' else 1
                emit_tile(rt, k7, eng, gsumA if eng == 'A' else gsumB,
                          cols[ei])
                cols[ei] += 1
                # stagger the scan step so each of its cross-engine hops
                # enters its FIFO with ~2 conv tiles of dep slack
                if k7 == 1 and t >= LAG:
                    emit_scan_a(t - LAG)
                if k7 == 4 and t >= LAG:
                    emit_scan_b(t - LAG)
            emit_finalize(t, gsumA, gsumB, cols)
        for t in range(T - LAG, T):
            emit_scan_a(t)
            emit_scan_b(t)

        nc.sync.dma_start(outh_d[:], hidh[:])

        for p in (tmp, ga_pool, fs, gs, spsum, psum, stackp, state, const):
            p.release()

    return nc


# -------------------------------------------------------------- host prep
def _prep_inputs(x, conv_w, conv_b, W1, b1, W2, b2, W3, b3):
    x = np.asarray(x, np.float32)
    conv_w = np.asarray(conv_w, np.float32)
    conv_b = np.asarray(conv_b, np.float32)

    xp = np.zeros((B, T, H + 2, W + 2, C), np.float32)
    xp[:, :, 1:H + 1, 1:W + 1, :] = x
    xin2 = np.empty((B, T, KP, JA, W), np.float32)
    rows = 2 * np.arange(JA)
    for c in range(3):
        for dx in range(3):
            for r in range(4):
                p = c * 12 + dx * 4 + r
                xin2[:, :, p] = np.moveaxis(
                    xp[:, :, rows + r, dx:dx + W, c], 0, 2)
    xin2 = xin2.reshape(B, T, KP, FREE).astype(ml_dtypes.bfloat16)

    smat = np.zeros((KP, 128), np.float32)
    for c in range(3):
        for dx in range(3):
            for r in range(4):
                p = c * 12 + dx * 4 + r
                for i in range(2):
                    dy = r - i
                    if 0 <= dy <= 2:
                        smat[p, i * F:(i + 1) * F] = conv_w[dy, dx, c, :]
    smat = smat.astype(ml_dtypes.bfloat16)
    cbias = np.concatenate([conv_b, conv_b]).reshape(M, 1).astype(np.float32)

    # gate weight layout [W1 | W2 | W3]; W1/W2 (sigmoid slots) halved for the
    # tanh-form sigmoid; wfeat rows also carry the GAP 1/(H*W).
    wfeat = np.zeros((M, 96), np.float32)
    whid = np.zeros((U, 96), np.float32)
    for g, Wg in enumerate([W1, W2, W3]):
        Wg = np.asarray(Wg, np.float32)
        half = 0.5 if g < 2 else 1.0
        for i in range(2):
            wfeat[i * F:(i + 1) * F, g * U:(g + 1) * U] = \
                Wg[0:F, :] * (half / float(H * W))
        whid[:, g * U:(g + 1) * U] = Wg[F:F + U, :] * half
    gbias = np.concatenate([
        np.asarray(b1, np.float32) * 0.5,
        np.asarray(b2, np.float32) * 0.5,
        np.asarray(b3, np.float32)]).reshape(1, 96)

    return (xin2, smat, cbias, wfeat.astype(ml_dtypes.bfloat16),
            whid.astype(ml_dtypes.bfloat16), gbias)


# ------------------------------------------------------------------ kernel
def kernel(x, conv_w, conv_b, W1, b1, W2, b2, W3, b3, W4, b4):
    global LAST_RESULTS
    xin2, smat, cbias, wfeat, whid, gbias = _prep_inputs(
        x, conv_w, conv_b, W1, b1, W2, b2, W3, b3)

    nc = _build_bass(use_gbias=bool(np.any(gbias)),
                     use_cbias=bool(np.any(cbias)))
    in_maps = [{
        'xin': np.ascontiguousarray(xin2[b]),
        'smat': smat,
        'cbias': cbias,
        'wfeat': wfeat,
        'whid': whid,
        'gbias': gbias,
    } for b in range(B)]

    res = run_bass_kernel_spmd(nc, in_maps, core_ids=list(range(B)))
    LAST_RESULTS = res
    out = np.stack([res.results[b]['outh'][0] for b in range(B)], axis=0)
    return (2.0 * out).astype(np.float32)


# revision 48
# speedup vs baseline: 2.2332x; 1.0497x over previous
"""Trainium2 Bass kernel for BasicCNN+LSTM (conv3x3+ReLU+GAP -> custom LSTM scan).

Self-contained: hardcodes shapes/sharding. Data-parallel over batch B=8 across
8 NeuronCores; each core processes one batch element end-to-end, the host
gathers the 8 [1,32] results.

Per-core device pipeline (per frame t of 24):
  - The frame's host-prepacked stack [36, 6272] bf16 is split by column into
    two SBUF bands (cols 0-3135 at partitions 0-35, rest at 64-99), so both
    PE row groups (h0/h64) work on ONE frame: the two matmuls of each tile
    run concurrently on the array and every LDWEIGHTS is pulled ahead during
    the other row group's stream (walrus' ldw-opt dedupe is broken, so the
    reload must be hidden, not removed).
  - Conv as K=36, M=128 matmuls (stationary zero-padded from 96 to 128 cols
    to trigger the compiler's fast-weight-load), N=448, into pool-rotated
    2-bank PSUM tiles [128, 1024] (bufs=3; per-tile pool rotation is what
    makes the WAR deps tile-granular - a single manually-slotted PSUM ring
    serialized everything through coarse subtile deps).
  - One fused ReLU(+bias)+GAP drain per tile over the [96, 2, 448] view,
    alternating engines per tile: ScalarE activation(Relu, accum_out) and
    VectorE tensor_scalar((x max 0) add 0, accum_out) - accum reduces with
    op1, so op1 must be the add; a single tensor source keeps full DVE rate.
  - GAP finalize (pairwise add tree -> bf16 fsum) on the otherwise idle
    GpSimd (Pool) engine; walrus rejects TensorScalarPtr on Pool, so only
    tensor_tensor/tcopy run there.
  - Scan step in tanh-only form: sigmoid(x) = (tanh(x/2)+1)/2 folded into
    host-halved W1/W2 slots; one Tanh over all 96 gate cols + one Tanh(cell)
    on ScalarE; the elementwise recurrence runs on GpSimd against a halved
    hidden state (host doubles the output), so it never head-of-line blocks
    conv drains on ACT/DVE; the step is emitted in two staggered halves so
    each cross-engine hop enters its FIFO with ~2 conv tiles of dep slack.
    Scan matmuls are bf16 (fp32 is 4 cycles/row on the PE). The reference's
    state-order swap bug is reproduced faithfully.

Known dead ends (measured): 3-slot drains on a 6-slot manual ring serialize
(subtile deps go coarse + only depth-2); matmuls into PE column-quadrant 3
(tile_position col 96) crash the exec unit (HW bug per docs); GpSimd has no
PSUM port; tensor_tensor_reduce fails walrus codegen.
"""
import sys
if '/opt/trn_rl_repo' not in sys.path:
    sys.path.insert(0, '/opt/trn_rl_repo')

import numpy as np
import ml_dtypes

import concourse.bass as bass
import concourse.mybir as mybir
import concourse.tile as tile
from concourse.vector_clock import ScopedClock
from concourse.bass_utils import run_bass_kernel_spmd

# ---------------------------------------------------------------- constants
B, T, H, W, C, F, U = 8, 24, 112, 112, 3, 48, 32
JA = 56            # vertical pixel-pair blocks (112 rows / 2)
KP = 36            # stack partitions: 3 c x 3 dx x 4 window rows
M = 96             # 2 pixels x 48 filters (real rows; stationary padded to 128)
NCHUNK = 14        # 448-col matmuls per frame
NQ = 448
FREE = JA * W      # stack free size per partition (elements)

FP32 = mybir.dt.float32
BF16 = mybir.dt.bfloat16

LAST_RESULTS = None  # BassKernelResults of the most recent run (for test.py)

# ------------------------------------------------- TileContext drain patch
# The container's walrus rejects >1 semaphore wait per instruction; Tile's
# kernel-tail drain aggregates all end-of-kernel waits onto one Drain.
# Spread them across single-wait NOPs on the sync engine instead.
def _patched_drain_and_barrier(self, tick_clock, wait_clock):
    nc = self.nc
    probe = nc.sync.nop(nofuse=True, hint="tail_waits")
    wait_clock.add_sem_waits(probe.ins, ScopedClock({None: tick_clock.global_clock}))
    waits = list(probe.ins.sync_info.on_wait or [])
    if len(waits) > 1:
        probe.ins.sync_info.on_wait = waits[:1]
        for i in range(1, len(waits)):
            extra = nc.sync.nop(nofuse=True, hint=f"tail_waits_{i}")
            si = extra.ins.sync_info
            if si is None:
                extra.ins.sync_info = mybir.SyncInfo(on_wait=[waits[i]], on_update=[])
            else:
                si.on_wait = [waits[i]]
    nc.sync.drain()
    nc.all_engine_barrier()
    popped = nc._tile_sem_poison_stack.pop()
    assert popped is self._sem_poison
    nc.clear_and_free_semaphores(list(self.sems.allocated().values()))
    nc.all_engine_barrier()


tile.TileContext._drain_and_barrier = _patched_drain_and_barrier

# Same walrus restriction for regular instructions: spill extra sem waits
# onto preceding same-engine NOPs at commit time.
_orig_commit = tile.TileContext._commit_instruction


def _patched_commit(self, inst, *args, **kwargs):
    si = getattr(inst, 'sync_info', None)
    if si is not None and si.on_wait and len(si.on_wait) > 1 \
            and inst.engine != mybir.EngineType.Unassigned:
        waits = list(si.on_wait)
        si.on_wait = waits[-1:]
        for w in waits[:-1]:
            nop = mybir.InstNoOp(
                name=self.nc.get_next_instruction_name(),
                ins=[], outs=[], bass_is_fusable=False)
            nop.engine = inst.engine
            nop.sync_info = mybir.SyncInfo(on_wait=[w], on_update=[])
            _orig_commit(self, nop, *args, **kwargs)
    return _orig_commit(self, inst, *args, **kwargs)


tile.TileContext._commit_instruction = _patched_commit

# NOTE: --enable-ldw-opt=true would dedupe the per-matmul stationary reloads,
# but this walrus build fails in visitInstLdweights with it enabled. Instead
# the matmul emission alternates PE row groups so each LDWEIGHTS is pulled
# ahead during the other group's stream.


# ------------------------------------------------------------- device code
def _build_bass(use_gbias=True, use_cbias=True):
    _build_bass.use_gbias = use_gbias
    _build_bass.use_cbias = use_cbias
    nc = bass.Bass('TRN2', target_bir_lowering=False, debug=False)

    xin = nc.dram_tensor('xin', [T, KP, FREE], BF16, kind='ExternalInput')
    smat_d = nc.dram_tensor('smat', [KP, 128], BF16, kind='ExternalInput')
    cbias_d = nc.dram_tensor('cbias', [M, 1], FP32, kind='ExternalInput')
    wfeat_d = nc.dram_tensor('wfeat', [M, 96], BF16, kind='ExternalInput')
    whid_d = nc.dram_tensor('whid', [U, 96], BF16, kind='ExternalInput')
    gbias_d = nc.dram_tensor('gbias', [1, 96], FP32, kind='ExternalInput')
    outh_d = nc.dram_tensor('outh', [1, U], FP32, kind='ExternalOutput')
    dbg_d = None
    if getattr(_build_bass, 'debug_fsums', False):
        dbg_d = nc.dram_tensor('dbg', [T, M], BF16, kind='ExternalOutput')

    Relu = mybir.ActivationFunctionType.Relu
    Tanh = mybir.ActivationFunctionType.Tanh
    Amax = mybir.AluOpType.max
    Aadd = mybir.AluOpType.add
    Amul = mybir.AluOpType.mult

    with tile.TileContext(nc) as tc:
        const = tc.alloc_tile_pool(name='const', bufs=1)
        state = tc.alloc_tile_pool(name='state', bufs=1)
        stackp = tc.alloc_tile_pool(name='stack', bufs=4)
        psum = tc.alloc_tile_pool(name='psum', bufs=3, space='PSUM')
        spsum = tc.alloc_tile_pool(name='spsum', bufs=2, space='PSUM')
        gs = tc.alloc_tile_pool(name='gs', bufs=4)
        fs = tc.alloc_tile_pool(name='fs', bufs=8)
        ga_pool = tc.alloc_tile_pool(name='ga', bufs=4)
        tmp = tc.alloc_tile_pool(name='tmp', bufs=8)

        # constants
        sc_all = const.tile([128, 128], BF16, tag='sc')
        for s in range(2):
            nc.sync.dma_start(sc_all[64 * s:64 * s + KP, :], smat_d[:])
        cbias = const.tile([M, 1], FP32, tag='cb')
        nc.sync.dma_start(cbias[:], cbias_d[:])
        wfeat = const.tile([M, 96], BF16, tag='wf')
        nc.sync.dma_start(wfeat[:], wfeat_d[:])
        whid = const.tile([U, 96], BF16, tag='wh')
        nc.sync.dma_start(whid[:], whid_d[:])
        gbias = const.tile([1, 96], FP32, tag='gb')
        nc.sync.dma_start(gbias[:], gbias_d[:])
        zt = const.tile([M, 3 * NQ], BF16, tag='zt')
        nc.vector.memset(zt[:], 0.0)
        ztv = zt.rearrange("p (r n) -> p r n", r=3)
        ones2 = const.tile([1, 2 * U], FP32, tag='ones2')
        nc.vector.memset(ones2[:], 1.0)
        halfs32 = const.tile([1, U], FP32, tag='halfs32')
        nc.vector.memset(halfs32[:], 0.5)

        # persistent scan state (hidden kept at half scale; host doubles out).
        # cellblk16 row 0 holds new_cell in bf16; a DVE 32x32 transpose gives
        # its partition-form in cpblk16's column 0 for the whid matmul.
        cellblk16 = state.tile([32, 32], BF16, tag='cellblk16')
        cpblk16 = state.tile([32, 32], BF16, tag='cpblk16')
        cellv = state.tile([1, U], FP32, tag='cellv')     # new_cell (fp32)
        cell_part = cpblk16[0:U, 0:1]                     # new_cell^T (bf16)
        hidh = state.tile([1, U], FP32, tag='hidh')       # 0.5 * new_hidden
        nc.vector.memset(cellblk16[:], 0.0)
        nc.vector.memset(cpblk16[:], 0.0)
        nc.vector.memset(cellv[:], 0.0)
        nc.vector.memset(hidh[:], 0.0)



        fsums = [None] * T

        frames = [None] * T

        def get_frame(t):
            # frame t split by column: cols 0-3135 -> band h0, rest -> h64
            if frames[t] is None:
                rt = stackp.tile([128, FREE // 2], BF16, tag='stk')
                nc.sync.dma_start(rt[0:KP, :], xin[t][:, 0:FREE // 2])
                nc.sync.dma_start(rt[64:64 + KP, :], xin[t][:, FREE // 2:])
                frames[t] = rt
            return frames[t]

        def emit_tile(rt, k7, eng, gsum, col):
            # one pool tile per 448-col chunk pair: band h0 chunk at
            # bank-aligned offset 0, band h64 chunk at 512; one drain
            ps = psum.tile([128, 1024], FP32, tag='ps', name='ps')
            for b in range(2):
                band = rt[64 * b:64 * b + KP, :]
                lhsT = sc_all[64 * b:64 * b + KP, :]
                nc.tensor.matmul(ps[:, b * 512:b * 512 + NQ], lhsT,
                                 band[:, k7 * NQ:(k7 + 1) * NQ],
                                 start=True, stop=True,
                                 tile_position=(64 * b, 0))
            psv = ps[0:M, :].rearrange("p (b n) -> p b n", b=2)[:, :, 0:NQ]
            if eng == 'A':
                nc.scalar.activation(psv, psv, Relu, bias=cbias[:],
                                     accum_out=gsum[:, col:col + 1])
            elif _build_bass.use_cbias:
                # (x + bias) max 0 with summing accum (two tensor sources)
                nc.vector.scalar_tensor_tensor(
                    out=psv, in0=psv, scalar=cbias[:], in1=ztv[:, 0:2, :],
                    op0=Aadd, op1=Amax, accum_out=gsum[:, col:col + 1])
            else:
                # zero bias: (x max 0) add 0; accum reduces with op1 (add),
                # single tensor source -> full DVE rate
                nc.vector.tensor_scalar(
                    out=psv, in0=psv, scalar1=0.0, scalar2=0.0,
                    op0=Amax, op1=Aadd, accum_out=gsum[:, col:col + 1])

        def emit_finalize(t, gsumA, gsumB, cols):
            # GAP finalize: pairwise add tree on the idle GpSimd -> bf16 fsum
            cs = [gsumA[:, c:c + 1] for c in range(cols[0])] + \
                 [gsumB[:, c:c + 1] for c in range(cols[1])]
            fsum = fs.tile([M, 1], BF16, tag='fsum', name='fsum')
            while len(cs) > 1:
                nxt = []
                for i in range(0, len(cs) - 1, 2):
                    o = fsum if len(cs) == 2 else \
                        tmp.tile([M, 1], FP32, tag='fst', name='fst')
                    nc.gpsimd.tensor_add(o[:], cs[i], cs[i + 1])
                    nxt.append(o)
                if len(cs) % 2:
                    nxt.append(cs[-1])
                cs = nxt
            fsums[t] = fsum
            if dbg_d is not None:
                nc.sync.dma_start(dbg_d[t], fsum[:])

        def emit_scan_a(t):
            # z-hidden part = prev new_cell (reference's state-order swap bug);
            # x1 multiplier = prev new_hidden (kept as hidh = hidden/2).
            fsum = fsums[t]
            pg = spsum.tile([1, 96], FP32, tag='sps')
            nc.tensor.matmul(pg[:], fsum[:], wfeat[:], start=True, stop=False)
            nc.tensor.matmul(pg[:], cell_part, whid[:], start=False, stop=True)
            if _build_bass.use_gbias:
                gpre = ga_pool.tile([1, 96], FP32, tag='gpre')
                nc.vector.tensor_add(gpre[:], pg[:], gbias[:])
            else:
                gpre = pg
            # gates in tanh form: W1/W2 slots were halved on the host, so
            # sigmoid(z@Wi) = (tanh(z@Wi/2)+1)/2 = (ga_i+1)/2.
            # elementwise recurrence on the otherwise idle GpSimd queue so it
            # never head-of-line blocks the conv drains on ACT/DVE
            ga = ga_pool.tile([1, 96], FP32, tag='ga')
            nc.scalar.activation(ga[:], gpre[:], Tanh)
            ua = tmp.tile([1, 2 * U], FP32, tag='ua')
            nc.gpsimd.tensor_add(ua[:], ga[:, 0:2 * U], ones2[:])
            u = tmp.tile([1, U], FP32, tag='u')
            nc.gpsimd.tensor_mul(u[:], ua[:, 0:U], hidh[:])  # = sig1*prev_hid
            vh = tmp.tile([1, U], FP32, tag='vh')
            nc.gpsimd.tensor_mul(vh[:], ua[:, U:2 * U], halfs32[:])
            v = tmp.tile([1, U], FP32, tag='v')
            nc.gpsimd.tensor_mul(v[:], vh[:], ga[:, 2 * U:3 * U])  # sig2*tanh3
            nc.gpsimd.tensor_add(cellv[:], v[:], u[:])    # new_cell

        def emit_scan_b(t):
            tcl = tmp.tile([1, U], FP32, tag='tcl')
            nc.scalar.activation(tcl[:], cellv[:], Tanh)
            ch = tmp.tile([1, U], FP32, tag='ch')
            nc.gpsimd.tensor_mul(ch[:], cellv[:], halfs32[:])
            nc.gpsimd.tensor_mul(hidh[:], ch[:], tcl[:])  # new_hidden / 2
            if t < T - 1:
                nc.gpsimd.tensor_copy(cellblk16[0:1, :], cellv[:])
                nc.vector.transpose(cpblk16[:], cellblk16[:])

        LAG = 6  # frames of lag between a frame's conv and its scan step
        for t in range(T):
            rt = get_frame(t)
            if t + 3 < T:
                get_frame(t + 3)  # prefetch 3 frames ahead
            if t + 2 < T:
                get_frame(t + 2)
            # alternate drain engines; flip per frame to balance 4/3 -> 3.5
            # ACT carries the 2 scan tanhs, so its optimal conv share is
            # ~3.31 tiles/frame: rotate [4,3,3] over 3 frames
            pat = ('ADADADA', 'DADADAD', 'DADADAD')[t % 3]
            gsumA = gs.tile([M, 4], FP32, tag='gsumA', name='gsumA')
            gsumB = gs.tile([M, 4], FP32, tag='gsumB', name='gsumB')
            cols = [0, 0]
            for k7 in range(7):
                eng = pat[k7]
                ei = 0 if eng == 'A' else 1
                emit_tile(rt, k7, eng, gsumA if eng == 'A' else gsumB,
                          cols[ei])
                cols[ei] += 1
                # stagger the scan step so each of its cross-engine hops
                # enters its FIFO with ~2 conv tiles of dep slack
                if k7 == 1 and t >= LAG:
                    emit_scan_a(t - LAG)
                if k7 == 4 and t >= LAG:
                    emit_scan_b(t - LAG)
            emit_finalize(t, gsumA, gsumB, cols)
        for t in range(T - LAG, T):
            emit_scan_a(t)
            emit_scan_b(t)

        nc.sync.dma_start(outh_d[:], hidh[:])

        for p in (tmp, ga_pool, fs, gs, spsum, psum, stackp, state, const):
            p.release()

    return nc


# -------------------------------------------------------------- host prep
def _prep_inputs(x, conv_w, conv_b, W1, b1, W2, b2, W3, b3):
    x = np.asarray(x, np.float32)
    conv_w = np.asarray(conv_w, np.float32)
    conv_b = np.asarray(conv_b, np.float32)

    xp = np.zeros((B, T, H + 2, W + 2, C), np.float32)
    xp[:, :, 1:H + 1, 1:W + 1, :] = x
    xin2 = np.empty((B, T, KP, JA, W), np.float32)
    rows = 2 * np.arange(JA)
    for c in range(3):
        for dx in range(3):
            for r in range(4):
                p = c * 12 + dx * 4 + r
                xin2[:, :, p] = np.moveaxis(
                    xp[:, :, rows + r, dx:dx + W, c], 0, 2)
    xin2 = xin2.reshape(B, T, KP, FREE).astype(ml_dtypes.bfloat16)

    smat = np.zeros((KP, 128), np.float32)
    for c in range(3):
        for dx in range(3):
            for r in range(4):
                p = c * 12 + dx * 4 + r
                for i in range(2):
                    dy = r - i
                    if 0 <= dy <= 2:
                        smat[p, i * F:(i + 1) * F] = conv_w[dy, dx, c, :]
    smat = smat.astype(ml_dtypes.bfloat16)
    cbias = np.concatenate([conv_b, conv_b]).reshape(M, 1).astype(np.float32)

    # gate weight layout [W1 | W2 | W3]; W1/W2 (sigmoid slots) halved for the
    # tanh-form sigmoid; wfeat rows also carry the GAP 1/(H*W).
    wfeat = np.zeros((M, 96), np.float32)
    whid = np.zeros((U, 96), np.float32)
    for g, Wg in enumerate([W1, W2, W3]):
        Wg = np.asarray(Wg, np.float32)
        half = 0.5 if g < 2 else 1.0
        for i in range(2):
            wfeat[i * F:(i + 1) * F, g * U:(g + 1) * U] = \
                Wg[0:F, :] * (half / float(H * W))
        whid[:, g * U:(g + 1) * U] = Wg[F:F + U, :] * half
    gbias = np.concatenate([
        np.asarray(b1, np.float32) * 0.5,
        np.asarray(b2, np.float32) * 0.5,
        np.asarray(b3, np.float32)]).reshape(1, 96)

    return (xin2, smat, cbias, wfeat.astype(ml_dtypes.bfloat16),
            whid.astype(ml_dtypes.bfloat16), gbias)


# ------------------------------------------------------------------ kernel
def kernel(x, conv_w, conv_b, W1, b1, W2, b2, W3, b3, W4, b4):
    global LAST_RESULTS
    xin2, smat, cbias, wfeat, whid, gbias = _prep_inputs(
        x, conv_w, conv_b, W1, b1, W2, b2, W3, b3)

    nc = _build_bass(use_gbias=bool(np.any(gbias)),
                     use_cbias=bool(np.any(cbias)))
    in_maps = [{
        'xin': np.ascontiguousarray(xin2[b]),
        'smat': smat,
        'cbias': cbias,
        'wfeat': wfeat,
        'whid': whid,
        'gbias': gbias,
    } for b in range(B)]

    res = run_bass_kernel_spmd(nc, in_maps, core_ids=list(range(B)))
    LAST_RESULTS = res
    out = np.stack([res.results[b]['outh'][0] for b in range(B)], axis=0)
    return (2.0 * out).astype(np.float32)
